# revision 35
# baseline (speedup 1.0000x reference)
"""DigitCaps dynamic-routing kernel for Trainium2 (8 NeuronCores, Bass/Tile).

Problem: B=256, IN_CAPS=3200, IN_DIM=8, OUT_CAPS=8, OUT_DIM=16, 3 routing
iterations.  Data-parallel over batch: 32 batches per core.

v2 design (per core):
  - batch processed in 8 "eighths" of 4 batches; per eighth, u_hat is
    created ONCE in SBUF bf16 in BOTH layouts and reused by both routing
    iterations (the baseline recreated it per iteration):
      u_jm [jm=128p, t, b, i]  via K=128 block-diagonal creation matmuls
      u_res [i=128p, t, b, jm] via XBAR DMA-transpose of u_jm tiles
  - x's 16x zero-padded block-diagonal operand is materialized ON CHIP:
    a resident xbz tile is memset to zero once, then only the true x
    bytes are DMA'd into the 16 diagonal bands per eighth (1.6 MB total
    instead of 52 MB of padded streams).
  - a-pass: stationary u_jm tile (128x128 bf16 -> hw fast-weight-load),
    moving block-diagonal v (8/16 cols) -> a^T [i-part, j] so softmax
    over j is a free-axis op on 128 partitions.
  - s-pass FLIPPED vs baseline: stationary u_res tile (FWL), moving
    c [i-part, 8j] -> 8-col matmuls accumulating s_jj' [jm, b, j'] over
    t; the j-diagonal is then extracted with a mask-multiply + reduce
    on all 128 partitions (replaces the baseline's 128-col matmuls and
    8-partition mask/ones matmul extraction).
  - PSUM->SBUF eviction of created u_hat round-robins over DVE /
    Activation / GPSIMD.
  - emission is software-pipelined across eighths so the PE keeps
    working (next eighth's creation) while squash chains run on vector.
"""

import sys

if "/opt/trn_rl_repo" not in sys.path:
    sys.path.insert(0, "/opt/trn_rl_repo")

import ml_dtypes
import numpy as np

import bass_rust
import concourse.bass as bass
import concourse.mybir as mybir
import concourse.tile as tile
from concourse._compat import with_exitstack
from concourse.bass_utils import run_bass_kernel_spmd
from concourse.vector_clock import ScopedClock

# ---------------------------------------------------------------------------
# Walrus on this toolchain rejects multi-wait CTRL instructions;
# TileContext's tail drain aggregates one wait per outstanding semaphore.
# Split the waits across consecutive SP drains.
_TILE_PATCHED = False


def _drain_and_barrier_split(self, tick_clock, wait_clock):
    drain_inst = self.nc.sync.drain()
    wait_clock.add_sem_waits(
        drain_inst.ins, ScopedClock({None: tick_clock.global_clock})
    )
    mi = drain_inst.ins
    waits = list(mi.sync_info.on_wait) if mi.sync_info else []
    if len(waits) > 1:
        si = mi.sync_info
        si.on_wait = waits[:1]
        mi.sync_info = si
        for i in range(1, len(waits)):
            extra = self.nc.sync.drain().ins
            extra.sync_info = bass_rust.SyncInfo(
                on_wait=waits[i : i + 1], on_update=[]
            )
    self.nc.all_engine_barrier()
    assert self.sems is not None
    popped = self.nc._tile_sem_poison_stack.pop()
    assert popped is self._sem_poison
    self.nc.clear_and_free_semaphores(list(self.sems.allocated().values()))
    self.nc.all_engine_barrier()


def _patch_tile():
    global _TILE_PATCHED
    if not _TILE_PATCHED:
        tile.TileContext._drain_and_barrier = _drain_and_barrier_split
        _TILE_PATCHED = True


_SW_COUNT = [0]


def _split_waits(nc):
    """This walrus build allows one sync wait per instruction: hoist extra
    waits onto same-engine NoOp carriers placed just before."""
    for f in nc.m.functions:
        for blk in f.blocks:
            insts = blk.instructions
            if not any(
                inst.sync_info and len(inst.sync_info.on_wait) > 1
                for inst in insts
            ):
                continue
            new = []
            for inst in insts:
                si = inst.sync_info
                waits = list(si.on_wait) if si else []
                if len(waits) > 1:
                    for w in waits[:-1]:
                        _SW_COUNT[0] += 1
                        car = mybir.InstNoOp(
                            name=f"I-sw{_SW_COUNT[0]}", engine=inst.engine
                        )
                        car.sync_info = bass_rust.SyncInfo(
                            on_wait=[w], on_update=[]
                        )
                        new.append(car)
                    si.on_wait = waits[-1:]
                    inst.sync_info = si
                new.append(inst)
            insts[:] = new


# ---------------------------------------------------------------------------
B, I, N, J, M = 256, 3200, 8, 8, 16
JM = J * M  # 128
N_CORES = 8
B_C = B // N_CORES  # 32
T = I // 128  # 25 i-tiles

IP = 16  # i's packed per K-chunk (K = IP*N = 128, uniform row group)
H = I // IP  # 200
CH_T = 128 // IP  # 8 creation chunks per 128-i tile

E = 4  # batches per eighth
NE = B_C // E  # 8 eighths

F32 = mybir.dt.float32
BF16 = mybir.dt.bfloat16


def _squash_chain(nc, small, ps, s_sb, ident, nb):
    """s_sb [128(jm), nb] f32 -> vT [nb, 128] f32.
    squash per capsule j: sq = sum_m s^2, v = sq*s/((1+sq)*sqrt(sq))."""
    sT_ps = ps.tile([nb, JM], F32, tag="sx")
    nc.tensor.matmul(sT_ps[:], s_sb[:], ident[:], is_transpose=True)
    sT = small.tile([nb, J, M], F32, tag="sT")
    nc.vector.tensor_copy(sT[:], sT_ps[:].rearrange("b (j m) -> b j m", m=M))
    s2 = small.tile([nb, J, M], F32, tag="s2")
    nc.vector.tensor_tensor(s2[:], sT[:], sT[:], mybir.AluOpType.mult)
    sq = small.tile([nb, J], F32, tag="sq")
    nc.vector.tensor_reduce(sq[:], s2[:], mybir.AxisListType.X, mybir.AluOpType.add)
    rt = small.tile([nb, J], F32, tag="rt")
    nc.scalar.activation(rt[:], sq[:], mybir.ActivationFunctionType.Sqrt)
    den = small.tile([nb, J], F32, tag="den")
    nc.vector.tensor_scalar_add(den[:], sq[:], 1.0)
    nc.vector.tensor_tensor(den[:], den[:], rt[:], mybir.AluOpType.mult)
    rden = small.tile([nb, J], F32, tag="rden")
    nc.vector.reciprocal(rden[:], den[:])
    scale = small.tile([nb, J], F32, tag="scale")
    nc.vector.tensor_tensor(scale[:], sq[:], rden[:], mybir.AluOpType.mult)
    vT = small.tile([nb, J, M], F32, tag="vT")
    scale_b = bass.AP(
        scale.tensor, scale[:].offset, [scale[:].ap[0], scale[:].ap[1], [0, M]]
    )
    nc.vector.tensor_tensor(vT[:], sT[:], scale_b, mybir.AluOpType.mult)
    return vT


def _vblk_from_vT(nc, small, vT, mask_rep, nb):
    """vT [nb, 128] f32 -> vblk [128(jm), nb, J] bf16 block-diagonal over j.
    Uses the XBAR transpose; only safe for nb >= 32."""
    vT16 = small.tile([nb, JM], BF16, tag="vT16")
    nc.vector.tensor_copy(vT16[:], vT[:])
    vjm = small.tile([JM, nb], BF16, tag="vjm")
    nc.sync.dma_start_transpose(vjm[:], vT16[:])
    vblk = small.tile([JM, nb, J], BF16, tag="vblk_tmp")
    vjm_b = bass.AP(vjm.tensor, vjm[:].offset, [vjm[:].ap[0], vjm[:].ap[1], [0, J]])
    mask_b = bass.AP(
        mask_rep.tensor,
        mask_rep[:].offset,
        [mask_rep[:].ap[0], [0, nb], mask_rep[:].ap[1]],
    )
    nc.vector.tensor_tensor(vblk[:], vjm_b, mask_b, mybir.AluOpType.mult)
    return vblk


@with_exitstack
def build_kernel(ctx, tc, outs, ins, reps=1, stage=3):
    nc = tc.nc
    (v_out,) = outs
    (wcr_d, xdg_d, xt_d, mask_d, ident_d) = ins

    TG = 8  # t-group size for batched softmax

    const = ctx.enter_context(tc.tile_pool(name="const", bufs=1))
    xbp = ctx.enter_context(tc.tile_pool(name="xbp", bufs=1))
    ujmp = ctx.enter_context(tc.tile_pool(name="ujmp", bufs=2))
    uresp = ctx.enter_context(tc.tile_pool(name="uresp", bufs=2))
    sm = ctx.enter_context(tc.tile_pool(name="sm", bufs=2))
    ctp = ctx.enter_context(tc.tile_pool(name="ctp", bufs=8))
    small = ctx.enter_context(tc.tile_pool(name="small", bufs=2))
    psq = ctx.enter_context(tc.tile_pool(name="psq", bufs=1, space="PSUM"))
    cpsp = ctx.enter_context(tc.tile_pool(name="cpsp", bufs=2, space="PSUM"))
    apsp = ctx.enter_context(tc.tile_pool(name="apsp", bufs=2, space="PSUM"))
    saccp = ctx.enter_context(tc.tile_pool(name="saccp", bufs=1, space="PSUM"))

    # Resident constants.  wcr/xt split into chunked DMAs so early matmuls
    # can start before the whole tensor lands.
    wcr = const.tile([128, H, JM], BF16)
    for c4 in range(4):
        nc.sync.dma_start(wcr[:, c4 * 50 : (c4 + 1) * 50, :],
                          wcr_d[:, c4 * 50 : (c4 + 1) * 50, :])
    xt = const.tile([128, H, B_C], BF16)
    nc.scalar.dma_start(xt[:], xt_d[:])
    mask_rep = const.tile([JM, J], BF16)
    nc.sync.dma_start(mask_rep[:], mask_d[:])
    maskT = const.tile([J, JM], BF16)
    nc.sync.dma_start(maskT[:], mask_d[:].rearrange("a b -> b a"))
    ones8 = const.tile([J, 1], BF16)
    nc.vector.memset(ones8[:], 1.0)
    ident = const.tile([128, 128], F32)
    nc.sync.dma_start(ident[:], ident_d[:])

    # Block-diagonal x operand: zeroed once, diagonal bands refilled per
    # eighth.  Layout [128(K rows = i16*8+n), ip, b, h]: h innermost keeps
    # each band refill DMA contiguous (8 descriptors, not a 2-byte scatter).
    xbz = const.tile([128, IP, E, H], BF16)
    nc.vector.memset(xbz[:, 0:8], 0.0)
    nc.gpsimd.memset(xbz[:, 8:16], 0.0)

    def fill_xbz(q):
        for c in range(IP):
            eng = nc.sync if c % 2 == 0 else nc.scalar
            eng.dma_start(
                xbz[c * N : (c + 1) * N, c, :, :], xdg_d[c, :, q, :, :]
            )

    def create(q, ujm, ures):
        """Creation matmuls + eviction + XBAR transpose for eighth q."""
        cps = None
        next_xbar = 0
        for t in range(T):
            t2 = t % 2
            if t2 == 0:
                cps = cpsp.tile([JM, 2, CH_T, IP, E], F32, tag="cps")
            for g in range(CH_T):
                h = t * CH_T + g
                nc.tensor.matmul(
                    cps[:, t2, g, :, :], wcr[:, h, :], xbz[:, :, :, h],
                    start=True, stop=True,
                )
            if t2 == 1 or t == T - 1:
                nt = t2 + 1
                dst = ujm[:, t - t2 : t + 1, :, :].rearrange(
                    "p tt b (g i) -> p tt g i b", i=IP
                )
                # 2:3 DVE:Act split of the eviction load
                if (t // 2) % 5 < 2:
                    nc.vector.tensor_copy(dst, cps[:, :nt])
                else:
                    nc.scalar.activation(
                        dst, cps[:, :nt], mybir.ActivationFunctionType.Copy
                    )
                # XBAR fully-evicted 5-blocks
                while next_xbar + 5 <= t + 1 or (t == T - 1 and next_xbar < T):
                    hi = min(next_xbar + 5, T)
                    eng = nc.sync if (next_xbar // 5) % 2 == 0 else nc.scalar
                    eng.dma_start_transpose(
                        ures[:, next_xbar:hi, :, :], ujm[:, next_xbar:hi, :, :]
                    )
                    next_xbar = hi

    def apass_softmax(q, it, ujm, vblk):
        """a-pass + batched softmax for eighth q, iteration it (2|3).
        Returns list of c_t tiles [128(i), TG, E, J] bf16 per t-group."""
        nslot = it - 1
        cts = []
        for g0 in range(0, T, TG):
            g1 = min(g0 + TG, T)
            ng = g1 - g0
            aps = apsp.tile([128, TG, E, 16], F32, tag="aps")
            for t in range(g0, g1):
                for b in range(E):
                    nc.tensor.matmul(
                        aps[:, t - g0, b, : nslot * J],
                        ujm[:, t, b, :],
                        vblk[:, q * E + b, :nslot, :],
                        start=True, stop=True,
                    )
            av = aps[:, :ng]
            e = sm.tile([128, TG, E, J], BF16, tag="e")
            if it == 2:
                # exp straight from PSUM; no logits copy needed
                nc.scalar.activation(
                    e[:, :ng], av[:, :, :, 0:J],
                    mybir.ActivationFunctionType.Exp,
                )
            else:
                lg0 = sm.tile([128, TG, E, J], F32, tag="lg0")
                nc.scalar.activation(
                    lg0[:, :ng], av[:, :, :, 0:J],
                    mybir.ActivationFunctionType.Copy,
                )
                lg = sm.tile([128, TG, E, J], F32, tag="lg")
                nc.vector.tensor_tensor(
                    lg[:, :ng], lg0[:, :ng], av[:, :, :, J : 2 * J],
                    mybir.AluOpType.add,
                )
                nc.scalar.activation(
                    e[:, :ng], lg[:, :ng], mybir.ActivationFunctionType.Exp
                )
            z = sm.tile([128, TG, E], F32, tag="z")
            nc.vector.tensor_reduce(
                z[:, :ng], e[:, :ng], mybir.AxisListType.X, mybir.AluOpType.add
            )
            rz = sm.tile([128, TG, E], F32, tag="rz")
            nc.vector.reciprocal(rz[:, :ng], z[:, :ng])
            c_t = ctp.tile([128, TG, E, J], BF16, tag="c_t")
            rzb = bass.AP(
                rz.tensor, rz[:, :ng].offset,
                [rz[:].ap[0], [rz[:].ap[1][0], ng], rz[:].ap[2], [0, J]],
            )
            nc.vector.tensor_tensor(
                c_t[:, :ng], e[:, :ng], rzb, mybir.AluOpType.mult
            )
            cts.append(c_t)
        return cts

    def spass(q, it, ures, cts):
        """s-pass: stationary c columns (cheap weight loads), moving u_res
        tiles; accumulates s_ps [J, E, JM] over t.  Returns s_sb [jm, E]."""
        sacc = saccp.tile([J, E, JM], F32, tag="sacc")
        nc.vector.memset(sacc[:], 0.0)
        for g0 in range(0, T, TG):
            c_t = cts[g0 // TG]
            for t in range(g0, min(g0 + TG, T)):
                for b in range(E):
                    nc.tensor.matmul(
                        sacc[:, b, :],
                        c_t[:, t - g0, b, :],
                        ures[:, t, b, :],
                        start=False, stop=False, skip_group_check=True,
                    )
        msb = small.tile([J, E, JM], BF16, tag="msb")
        maskT_b = bass.AP(
            maskT.tensor, maskT[:].offset,
            [maskT[:].ap[0], [0, E], maskT[:].ap[1]],
        )
        nc.vector.tensor_tensor(msb[:], sacc[:], maskT_b, mybir.AluOpType.mult)
        s2_ps = psq.tile([JM, E], F32, tag="sx")
        for b in range(E):
            nc.tensor.matmul(
                s2_ps[:, b : b + 1], msb[:, b, :], ones8[:],
                start=True, stop=True,
            )
        s_sb = small.tile([JM, E], F32, tag="s_sb")
        nc.vector.tensor_copy(s_sb[:], s2_ps[:])
        return s_sb

    def vblk_write(q, vTh, vblk):
        """vTh [E, 128] f32 -> vblk[:, q*E:(q+1)*E, 1, :] via PE transpose."""
        vps = psq.tile([JM, E], F32, tag="sx")
        nc.tensor.matmul(
            vps[:], vTh[:], ident[0:E, 0:E], is_transpose=True
        )
        vjm = small.tile([JM, E], BF16, tag="vjms")
        nc.vector.tensor_copy(vjm[:], vps[:])
        vjm_b = bass.AP(
            vjm.tensor, vjm[:].offset, [vjm[:].ap[0], vjm[:].ap[1], [0, J]]
        )
        mask_b = bass.AP(
            mask_rep.tensor, mask_rep[:].offset,
            [mask_rep[:].ap[0], [0, E], mask_rep[:].ap[1]],
        )
        nc.vector.tensor_tensor(
            vblk[:, q * E : (q + 1) * E, 1, :], vjm_b, mask_b,
            mybir.AluOpType.mult,
        )

    vblk = const.tile([JM, B_C, 2, J], BF16, tag="vblk")

    for rep in range(reps):
        fill_xbz(0)
        # ---- iteration 1 (all batches): s1 = (1/8) sum_(i,n) W x ----------
        s1_ps = psq.tile([JM, B_C], F32, tag="sx")
        for h in range(H):
            nc.tensor.matmul(
                s1_ps[:], wcr[:, h, :], xt[:, h, :],
                start=(h == 0), stop=(h == H - 1),
            )
        s_sb = small.tile([JM, B_C], F32, tag="s_all")
        nc.vector.tensor_scalar_mul(s_sb[:], s1_ps[:], 1.0 / J)
        vT = _squash_chain(nc, small, psq, s_sb, ident, B_C)
        vb = _vblk_from_vT(nc, small, vT, mask_rep, B_C)
        nc.vector.tensor_copy(vblk[:, :, 0, :], vb[:])

        ujm = [None] * NE
        ures = [None] * NE

        def mk_u(q):
            uj = ujmp.tile([JM, T, E, 128], BF16, tag="ujm")
            ur = uresp.tile([128, T, E, JM], BF16, tag="ures")
            ujm[q] = uj
            ures[q] = ur

        mk_u(0)
        create(0, ujm[0], ures[0])
        if stage < 3:
            # ablation: creation only
            for q in range(1, NE):
                fill_xbz(q)
                mk_u(q)
                create(q, ujm[q], ures[q])
            nc.sync.dma_start(
                v_out[:].rearrange("b j m -> b (j m)")[:, :], vT[:]
            )
            continue

        cts2 = apass_softmax(0, 2, ujm[0], vblk)
        for q in range(NE):
            if q + 1 < NE:
                fill_xbz(q + 1)
            # it2 finish: s-pass, squash, vblk slot 1
            s_sb2 = spass(q, 2, ures[q], cts2)
            vT2 = _squash_chain(nc, small, psq, s_sb2, ident, E)
            vblk_write(q, vT2, vblk)
            # next eighth's creation fills the PE while squash2's vector
            # chain and vblk broadcast complete
            if q + 1 < NE:
                mk_u(q + 1)
                create(q + 1, ujm[q + 1], ures[q + 1])
            # it3
            cts3 = apass_softmax(q, 3, ujm[q], vblk)
            if q + 1 < NE:
                cts2 = apass_softmax(q + 1, 2, ujm[q + 1], vblk)
            s_sb3 = spass(q, 3, ures[q], cts3)
            vT3 = _squash_chain(nc, small, psq, s_sb3, ident, E)
            nc.sync.dma_start(
                v_out[:].rearrange("b j m -> b (j m)")[q * E : (q + 1) * E, :],
                vT3[:],
            )


_NC_CACHE = {}


def _build_nc(reps=1, stage=3):
    key = (reps, stage)
    if key not in _NC_CACHE:
        _patch_tile()
        nc = bass.Bass("TRN2", target_bir_lowering=False, debug=False)
        wcr_d = nc.dram_tensor("wcr", [128, H, JM], BF16, kind="ExternalInput").ap()
        xdg_d = nc.dram_tensor(
            "xdg", [IP, N, NE, E, H], BF16, kind="ExternalInput"
        ).ap()
        xt_d = nc.dram_tensor("xt", [128, H, B_C], BF16, kind="ExternalInput").ap()
        mask_d = nc.dram_tensor("mask", [JM, J], BF16, kind="ExternalInput").ap()
        ident_d = nc.dram_tensor("ident", [128, 128], F32, kind="ExternalInput").ap()
        v_d = nc.dram_tensor("v", [B_C, J, M], F32, kind="ExternalOutput").ap()
        with tile.TileContext(nc) as tc:
            build_kernel(
                tc,
                [v_d],
                [wcr_d, xdg_d, xt_d, mask_d, ident_d],
                reps=reps,
                stage=stage,
            )
        _split_waits(nc)
        _NC_CACHE[key] = nc
    return _NC_CACHE[key]


def host_prep(x, W):
    """Returns (wcr, xdg_all, xt_all, mask, ident); x-deriveds cover all B.
    Row order of the 128 K-rows is (i16, n): i = h*IP + i16."""
    bf = ml_dtypes.bfloat16
    nb = x.shape[0]
    # wcr[(i16*N + n), h, jm] = W[h*IP + i16, j, n, m]
    Wr = np.ascontiguousarray(W.transpose(0, 2, 1, 3)).reshape(I, N, JM)
    Wr = Wr.reshape(H, IP, N, JM)
    wcr = np.ascontiguousarray(Wr.transpose(1, 2, 0, 3)).reshape(128, H, JM)
    # x rows in the same (i16, n) order per h
    xr = x.reshape(nb, H, IP, N)
    xrows = np.ascontiguousarray(xr.transpose(2, 3, 1, 0)).reshape(128, H, nb)
    # xdg[c, n, b, h] = x[b, h*16+c, n]  (diagonal bands, true bytes only)
    xdg = np.ascontiguousarray(xr.transpose(2, 3, 0, 1))  # [IP, N, nb, H]
    mask = np.zeros((JM, J), np.float32)
    for j in range(J):
        mask[j * M : (j + 1) * M, j] = 1.0
    ident = np.eye(128, dtype=np.float32)
    return (
        wcr.astype(bf),
        xdg.astype(bf),
        xrows.astype(bf),
        mask.astype(bf),
        ident,
    )


def core_in_maps(x, W):
    """Per-core input dicts for run_bass_kernel_spmd."""
    wcr, xdg_all, xt_all, mask, ident = host_prep(x, W)
    in_maps = []
    for c in range(N_CORES):
        bs = slice(c * B_C, (c + 1) * B_C)
        # [IP, N, 32, H] -> [IP, N, NE, E, H]
        xdg_c = np.ascontiguousarray(
            xdg_all[:, :, bs, :].reshape(IP, N, NE, E, H)
        )
        in_maps.append(
            {
                "wcr": wcr,
                "xdg": xdg_c,
                "xt": np.ascontiguousarray(xt_all[:, :, bs]),
                "mask": mask,
                "ident": ident,
            }
        )
    return in_maps


def kernel(x, W):
    x = np.asarray(x, np.float32)
    W = np.asarray(W, np.float32)
    in_maps = core_in_maps(x, W)
    nc = _build_nc()
    res = run_bass_kernel_spmd(nc, in_maps, list(range(N_CORES)))
    out = np.concatenate([res.results[c]["v"] for c in range(N_CORES)], axis=0)
    return out.astype(np.float32)


# revision 45
# speedup vs baseline: 1.2165x; 1.2165x over previous
"""DigitCaps dynamic-routing kernel for Trainium2 (8 NeuronCores, Bass/Tile).

Problem: B=256, IN_CAPS=3200, IN_DIM=8, OUT_CAPS=8, OUT_DIM=16, 3 routing
iterations.  Data-parallel over batch: 32 batches per core.

v2 design (per core):
  - batch processed in 8 "eighths" of 4 batches; per eighth, u_hat is
    created ONCE in SBUF bf16 in BOTH layouts and reused by both routing
    iterations (the baseline recreated it per iteration):
      u_jm [jm=128p, t, b, i]  via K=128 block-diagonal creation matmuls
      u_res [i=128p, t, b, jm] via XBAR DMA-transpose of u_jm tiles
  - x's 16x zero-padded block-diagonal operand is materialized ON CHIP:
    a resident xbz tile is memset to zero once, then only the true x
    bytes are DMA'd into the 16 diagonal bands per eighth (1.6 MB total
    instead of 52 MB of padded streams).
  - a-pass: stationary u_jm tile (128x128 bf16), moving block-diagonal
    v (8/16 cols) -> a^T [i-part, j] so softmax over j is a free-axis
    op on 128 partitions; softmax batched over 8-tile groups.
  - s-pass: stationary c columns (cheap 8-col weight loads), moving
    u_res tiles, accumulating s_ps [J, b, jm] over t with a mask/ones
    extraction (baseline orientation).
  - PSUM->SBUF eviction of created u_hat in 2-tile batches split 2:3
    over DVE / Activation.
  - emission is software-pipelined across eighths so the PE keeps
    working (next eighth's creation) while squash chains run on vector.
"""

import sys

if "/opt/trn_rl_repo" not in sys.path:
    sys.path.insert(0, "/opt/trn_rl_repo")

import ml_dtypes
import numpy as np

import bass_rust
import concourse.bass as bass
import concourse.mybir as mybir
import concourse.tile as tile
from concourse._compat import with_exitstack
from concourse.bass_utils import run_bass_kernel_spmd
from concourse.vector_clock import ScopedClock

# ---------------------------------------------------------------------------
# Walrus on this toolchain rejects multi-wait CTRL instructions;
# TileContext's tail drain aggregates one wait per outstanding semaphore.
# Split the waits across consecutive SP drains.
_TILE_PATCHED = False


def _drain_and_barrier_split(self, tick_clock, wait_clock):
    drain_inst = self.nc.sync.drain()
    wait_clock.add_sem_waits(
        drain_inst.ins, ScopedClock({None: tick_clock.global_clock})
    )
    mi = drain_inst.ins
    waits = list(mi.sync_info.on_wait) if mi.sync_info else []
    if len(waits) > 1:
        si = mi.sync_info
        si.on_wait = waits[:1]
        mi.sync_info = si
        for i in range(1, len(waits)):
            extra = self.nc.sync.drain().ins
            extra.sync_info = bass_rust.SyncInfo(
                on_wait=waits[i : i + 1], on_update=[]
            )
    self.nc.all_engine_barrier()
    assert self.sems is not None
    popped = self.nc._tile_sem_poison_stack.pop()
    assert popped is self._sem_poison
    self.nc.clear_and_free_semaphores(list(self.sems.allocated().values()))
    self.nc.all_engine_barrier()


def _patch_tile():
    global _TILE_PATCHED
    if not _TILE_PATCHED:
        tile.TileContext._drain_and_barrier = _drain_and_barrier_split
        _TILE_PATCHED = True


_SW_COUNT = [0]


def _split_waits(nc):
    """This walrus build allows one sync wait per instruction: hoist extra
    waits onto same-engine NoOp carriers placed just before."""
    for f in nc.m.functions:
        for blk in f.blocks:
            insts = blk.instructions
            if not any(
                inst.sync_info and len(inst.sync_info.on_wait) > 1
                for inst in insts
            ):
                continue
            new = []
            for inst in insts:
                si = inst.sync_info
                waits = list(si.on_wait) if si else []
                if len(waits) > 1:
                    for w in waits[:-1]:
                        _SW_COUNT[0] += 1
                        car = mybir.InstNoOp(
                            name=f"I-sw{_SW_COUNT[0]}", engine=inst.engine
                        )
                        car.sync_info = bass_rust.SyncInfo(
                            on_wait=[w], on_update=[]
                        )
                        new.append(car)
                    si.on_wait = waits[-1:]
                    inst.sync_info = si
                new.append(inst)
            insts[:] = new


# ---------------------------------------------------------------------------
B, I, N, J, M = 256, 3200, 8, 8, 16
JM = J * M  # 128
N_CORES = 8
B_C = B // N_CORES  # 32
T = I // 128  # 25 i-tiles

IP = 16  # i's packed per K-chunk (K = IP*N = 128, uniform row group)
H = I // IP  # 200
CH_T = 128 // IP  # 8 creation chunks per 128-i tile

E = 4  # batches per eighth
NE = B_C // E  # 8 eighths

F32 = mybir.dt.float32
BF16 = mybir.dt.bfloat16


def _squash_chain(nc, small, ps, s_sb, ident, nb):
    """s_sb [128(jm), nb] f32 -> vT [nb, 128] f32.
    squash per capsule j: sq = sum_m s^2, v = sq*s/((1+sq)*sqrt(sq))."""
    sT_ps = ps.tile([nb, JM], F32, tag="sx")
    nc.tensor.matmul(sT_ps[:], s_sb[:], ident[:], is_transpose=True)
    sT = small.tile([nb, J, M], F32, tag="sT")
    nc.vector.tensor_copy(sT[:], sT_ps[:].rearrange("b (j m) -> b j m", m=M))
    s2 = small.tile([nb, J, M], F32, tag="s2")
    nc.vector.tensor_tensor(s2[:], sT[:], sT[:], mybir.AluOpType.mult)
    sq = small.tile([nb, J], F32, tag="sq")
    nc.vector.tensor_reduce(sq[:], s2[:], mybir.AxisListType.X, mybir.AluOpType.add)
    rt = small.tile([nb, J], F32, tag="rt")
    nc.scalar.activation(rt[:], sq[:], mybir.ActivationFunctionType.Sqrt)
    den = small.tile([nb, J], F32, tag="den")
    nc.vector.tensor_scalar_add(den[:], sq[:], 1.0)
    nc.vector.tensor_tensor(den[:], den[:], rt[:], mybir.AluOpType.mult)
    rden = small.tile([nb, J], F32, tag="rden")
    nc.vector.reciprocal(rden[:], den[:])
    scale = small.tile([nb, J], F32, tag="scale")
    nc.vector.tensor_tensor(scale[:], sq[:], rden[:], mybir.AluOpType.mult)
    vT = small.tile([nb, J, M], F32, tag="vT")
    scale_b = bass.AP(
        scale.tensor, scale[:].offset, [scale[:].ap[0], scale[:].ap[1], [0, M]]
    )
    nc.vector.tensor_tensor(vT[:], sT[:], scale_b, mybir.AluOpType.mult)
    return vT


def _vblk_from_vT(nc, small, vT, mask_rep, nb):
    """vT [nb, 128] f32 -> vblk [128(jm), nb, J] bf16 block-diagonal over j.
    Uses the XBAR transpose; only safe for nb >= 32."""
    vT16 = small.tile([nb, JM], BF16, tag="vT16")
    nc.vector.tensor_copy(vT16[:], vT[:])
    vjm = small.tile([JM, nb], BF16, tag="vjm")
    nc.sync.dma_start_transpose(vjm[:], vT16[:])
    vblk = small.tile([JM, nb, J], BF16, tag="vblk_tmp")
    vjm_b = bass.AP(vjm.tensor, vjm[:].offset, [vjm[:].ap[0], vjm[:].ap[1], [0, J]])
    mask_b = bass.AP(
        mask_rep.tensor,
        mask_rep[:].offset,
        [mask_rep[:].ap[0], [0, nb], mask_rep[:].ap[1]],
    )
    nc.vector.tensor_tensor(vblk[:], vjm_b, mask_b, mybir.AluOpType.mult)
    return vblk


@with_exitstack
def build_kernel(ctx, tc, outs, ins, reps=1, stage=3):
    nc = tc.nc
    (v_out,) = outs
    (wcr_d, xdg_d, xt_d, mask_d, ident_d) = ins

    TG = 8  # t-group size for batched softmax

    const = ctx.enter_context(tc.tile_pool(name="const", bufs=1))
    xbp = ctx.enter_context(tc.tile_pool(name="xbp", bufs=1))
    ujmp = ctx.enter_context(tc.tile_pool(name="ujmp", bufs=2))
    uresp = ctx.enter_context(tc.tile_pool(name="uresp", bufs=2))
    sm = ctx.enter_context(tc.tile_pool(name="sm", bufs=2))
    ctp = ctx.enter_context(tc.tile_pool(name="ctp", bufs=8))
    small = ctx.enter_context(tc.tile_pool(name="small", bufs=2))
    psq = ctx.enter_context(tc.tile_pool(name="psq", bufs=1, space="PSUM"))
    cpsp = ctx.enter_context(tc.tile_pool(name="cpsp", bufs=2, space="PSUM"))
    apsp = ctx.enter_context(tc.tile_pool(name="apsp", bufs=2, space="PSUM"))
    saccp = ctx.enter_context(tc.tile_pool(name="saccp", bufs=1, space="PSUM"))

    # Resident constants.  wcr/xt split into chunked DMAs so early matmuls
    # can start before the whole tensor lands.
    wcr = const.tile([128, H, JM], BF16)
    for c4 in range(4):
        nc.sync.dma_start(wcr[:, c4 * 50 : (c4 + 1) * 50, :],
                          wcr_d[:, c4 * 50 : (c4 + 1) * 50, :])
    xt = const.tile([128, H, B_C], BF16)
    nc.scalar.dma_start(xt[:], xt_d[:])
    mask_rep = const.tile([JM, J], BF16)
    nc.sync.dma_start(mask_rep[:], mask_d[:])
    maskT = const.tile([J, JM], BF16)
    nc.sync.dma_start(maskT[:], mask_d[:].rearrange("a b -> b a"))
    ones8 = const.tile([J, 1], BF16)
    nc.vector.memset(ones8[:], 1.0)
    ident = const.tile([128, 128], F32)
    nc.sync.dma_start(ident[:], ident_d[:])

    # Block-diagonal x operand: zeroed once, diagonal bands refilled per
    # eighth.  Layout [128(K rows = i16*8+n), ip, b, h]: h innermost keeps
    # each band refill DMA contiguous (8 descriptors, not a 2-byte scatter).
    xbz = const.tile([128, IP, E, H], BF16)
    nc.vector.memset(xbz[:, 0:8], 0.0)
    nc.gpsimd.memset(xbz[:, 8:16], 0.0)

    def fill_xbz(q):
        for c in range(IP):
            eng = nc.sync if c % 2 == 0 else nc.scalar
            eng.dma_start(
                xbz[c * N : (c + 1) * N, c, :, :], xdg_d[c, :, q, :, :]
            )

    def create(q, ujm, ures):
        """Creation matmuls + eviction + XBAR transpose for eighth q."""
        cps = None
        next_xbar = 0
        for t in range(T):
            t2 = t % 2
            if t2 == 0:
                cps = cpsp.tile([JM, 2, CH_T, IP, E], F32, tag="cps")
            for g in range(CH_T):
                h = t * CH_T + g
                nc.tensor.matmul(
                    cps[:, t2, g, :, :], wcr[:, h, :], xbz[:, :, :, h],
                    start=True, stop=True,
                )
            if t2 == 1 or t == T - 1:
                nt = t2 + 1
                dst = ujm[:, t - t2 : t + 1, :, :].rearrange(
                    "p tt b (g i) -> p tt g i b", i=IP
                )
                # 2:3 DVE:Act split of the eviction load
                if (t // 2) % 5 < 2:
                    nc.vector.tensor_copy(dst, cps[:, :nt])
                else:
                    nc.scalar.activation(
                        dst, cps[:, :nt], mybir.ActivationFunctionType.Copy
                    )
                # XBAR fully-evicted 5-blocks
                while next_xbar + 5 <= t + 1 or (t == T - 1 and next_xbar < T):
                    hi = min(next_xbar + 5, T)
                    eng = nc.sync if (next_xbar // 5) % 2 == 0 else nc.scalar
                    eng.dma_start_transpose(
                        ures[:, next_xbar:hi, :, :], ujm[:, next_xbar:hi, :, :]
                    )
                    next_xbar = hi

    def apass_softmax(q, it, ujm, vblk):
        """a-pass + batched softmax for eighth q, iteration it (2|3).
        Returns list of c_t tiles [128(i), TG, E, J] bf16 per t-group."""
        nslot = it - 1
        cts = []
        for g0 in range(0, T, TG):
            g1 = min(g0 + TG, T)
            ng = g1 - g0
            aps = apsp.tile([128, TG, E, 16], F32, tag="aps")
            for t in range(g0, g1):
                for b in range(E):
                    nc.tensor.matmul(
                        aps[:, t - g0, b, : nslot * J],
                        ujm[:, t, b, :],
                        vblk[:, q * E + b, :nslot, :],
                        start=True, stop=True,
                    )
            av = aps[:, :ng]
            e = sm.tile([128, TG, E, J], BF16, tag="e")
            if it == 2:
                # exp straight from PSUM; no logits copy needed
                nc.scalar.activation(
                    e[:, :ng], av[:, :, :, 0:J],
                    mybir.ActivationFunctionType.Exp,
                )
            else:
                lg0 = sm.tile([128, TG, E, J], F32, tag="lg0")
                nc.scalar.activation(
                    lg0[:, :ng], av[:, :, :, 0:J],
                    mybir.ActivationFunctionType.Copy,
                )
                lg = sm.tile([128, TG, E, J], F32, tag="lg")
                nc.vector.tensor_tensor(
                    lg[:, :ng], lg0[:, :ng], av[:, :, :, J : 2 * J],
                    mybir.AluOpType.add,
                )
                nc.scalar.activation(
                    e[:, :ng], lg[:, :ng], mybir.ActivationFunctionType.Exp
                )
            z = sm.tile([128, TG, E], F32, tag="z")
            nc.vector.tensor_reduce(
                z[:, :ng], e[:, :ng], mybir.AxisListType.X, mybir.AluOpType.add
            )
            rz = sm.tile([128, TG, E], F32, tag="rz")
            nc.vector.reciprocal(rz[:, :ng], z[:, :ng])
            c_t = ctp.tile([128, TG, E, J], BF16, tag="c_t")
            rzb = bass.AP(
                rz.tensor, rz[:, :ng].offset,
                [rz[:].ap[0], [rz[:].ap[1][0], ng], rz[:].ap[2], [0, J]],
            )
            nc.vector.tensor_tensor(
                c_t[:, :ng], e[:, :ng], rzb, mybir.AluOpType.mult
            )
            cts.append(c_t)
        return cts

    def spass(q, it, ures, cts):
        """s-pass: stationary c columns (cheap weight loads), moving u_res
        tiles; accumulates s_ps [J, E, JM] over t.  Returns s_sb [jm, E]."""
        sacc = saccp.tile([J, E, JM], F32, tag="sacc")
        nc.vector.memset(sacc[:], 0.0)
        for g0 in range(0, T, TG):
            c_t = cts[g0 // TG]
            for t in range(g0, min(g0 + TG, T)):
                for b in range(E):
                    nc.tensor.matmul(
                        sacc[:, b, :],
                        c_t[:, t - g0, b, :],
                        ures[:, t, b, :],
                        start=False, stop=False, skip_group_check=True,
                    )
        msb = small.tile([J, E, JM], BF16, tag="msb")
        maskT_b = bass.AP(
            maskT.tensor, maskT[:].offset,
            [maskT[:].ap[0], [0, E], maskT[:].ap[1]],
        )
        nc.vector.tensor_tensor(msb[:], sacc[:], maskT_b, mybir.AluOpType.mult)
        s2_ps = psq.tile([JM, E], F32, tag="sx")
        for b in range(E):
            nc.tensor.matmul(
                s2_ps[:, b : b + 1], msb[:, b, :], ones8[:],
                start=True, stop=True,
            )
        s_sb = small.tile([JM, E], F32, tag="s_sb")
        nc.vector.tensor_copy(s_sb[:], s2_ps[:])
        return s_sb

    def vblk_write(q, vTh, vblk):
        """vTh [E, 128] f32 -> vblk[:, q*E:(q+1)*E, 1, :] via PE transpose."""
        vps = psq.tile([JM, E], F32, tag="sx")
        nc.tensor.matmul(
            vps[:], vTh[:], ident[0:E, 0:E], is_transpose=True
        )
        vjm = small.tile([JM, E], BF16, tag="vjms")
        nc.vector.tensor_copy(vjm[:], vps[:])
        vjm_b = bass.AP(
            vjm.tensor, vjm[:].offset, [vjm[:].ap[0], vjm[:].ap[1], [0, J]]
        )
        mask_b = bass.AP(
            mask_rep.tensor, mask_rep[:].offset,
            [mask_rep[:].ap[0], [0, E], mask_rep[:].ap[1]],
        )
        nc.vector.tensor_tensor(
            vblk[:, q * E : (q + 1) * E, 1, :], vjm_b, mask_b,
            mybir.AluOpType.mult,
        )

    vblk = const.tile([JM, B_C, 2, J], BF16, tag="vblk")

    for rep in range(reps):
        fill_xbz(0)
        # ---- iteration 1 (all batches): s1 = (1/8) sum_(i,n) W x ----------
        s1_ps = psq.tile([JM, B_C], F32, tag="sx")
        for h in range(H):
            nc.tensor.matmul(
                s1_ps[:], wcr[:, h, :], xt[:, h, :],
                start=(h == 0), stop=(h == H - 1),
            )
        s_sb = small.tile([JM, B_C], F32, tag="s_all")
        nc.vector.tensor_scalar_mul(s_sb[:], s1_ps[:], 1.0 / J)
        vT = _squash_chain(nc, small, psq, s_sb, ident, B_C)
        vb = _vblk_from_vT(nc, small, vT, mask_rep, B_C)
        nc.vector.tensor_copy(vblk[:, :, 0, :], vb[:])

        ujm = [None] * NE
        ures = [None] * NE

        def mk_u(q):
            uj = ujmp.tile([JM, T, E, 128], BF16, tag="ujm")
            ur = uresp.tile([128, T, E, JM], BF16, tag="ures")
            ujm[q] = uj
            ures[q] = ur

        mk_u(0)
        create(0, ujm[0], ures[0])
        if stage < 3:
            # ablation: creation only
            for q in range(1, NE):
                fill_xbz(q)
                mk_u(q)
                create(q, ujm[q], ures[q])
            nc.sync.dma_start(
                v_out[:].rearrange("b j m -> b (j m)")[:, :], vT[:]
            )
            continue

        cts2 = apass_softmax(0, 2, ujm[0], vblk)
        for q in range(NE):
            if q + 1 < NE:
                fill_xbz(q + 1)
            # it2 finish: s-pass, squash, vblk slot 1
            s_sb2 = spass(q, 2, ures[q], cts2)
            vT2 = _squash_chain(nc, small, psq, s_sb2, ident, E)
            vblk_write(q, vT2, vblk)
            # next eighth's creation fills the PE while squash2's vector
            # chain and vblk broadcast complete
            if q + 1 < NE:
                mk_u(q + 1)
                create(q + 1, ujm[q + 1], ures[q + 1])
            # it3
            cts3 = apass_softmax(q, 3, ujm[q], vblk)
            if q + 1 < NE:
                cts2 = apass_softmax(q + 1, 2, ujm[q + 1], vblk)
            s_sb3 = spass(q, 3, ures[q], cts3)
            vT3 = _squash_chain(nc, small, psq, s_sb3, ident, E)
            nc.sync.dma_start(
                v_out[:].rearrange("b j m -> b (j m)")[q * E : (q + 1) * E, :],
                vT3[:],
            )


_NC_CACHE = {}


def _build_nc(reps=1, stage=3):
    key = (reps, stage)
    if key not in _NC_CACHE:
        _patch_tile()
        nc = bass.Bass("TRN2", target_bir_lowering=False, debug=False)
        wcr_d = nc.dram_tensor("wcr", [128, H, JM], BF16, kind="ExternalInput").ap()
        xdg_d = nc.dram_tensor(
            "xdg", [IP, N, NE, E, H], BF16, kind="ExternalInput"
        ).ap()
        xt_d = nc.dram_tensor("xt", [128, H, B_C], BF16, kind="ExternalInput").ap()
        mask_d = nc.dram_tensor("mask", [JM, J], BF16, kind="ExternalInput").ap()
        ident_d = nc.dram_tensor("ident", [128, 128], F32, kind="ExternalInput").ap()
        v_d = nc.dram_tensor("v", [B_C, J, M], F32, kind="ExternalOutput").ap()
        with tile.TileContext(nc) as tc:
            build_kernel(
                tc,
                [v_d],
                [wcr_d, xdg_d, xt_d, mask_d, ident_d],
                reps=reps,
                stage=stage,
            )
        _split_waits(nc)
        _NC_CACHE[key] = nc
    return _NC_CACHE[key]


def host_prep(x, W):
    """Returns (wcr, xdg_all, xt_all, mask, ident); x-deriveds cover all B.
    Row order of the 128 K-rows is (i16, n): i = h*IP + i16."""
    bf = ml_dtypes.bfloat16
    nb = x.shape[0]
    # wcr[(i16*N + n), h, jm] = W[h*IP + i16, j, n, m]
    Wr = np.ascontiguousarray(W.transpose(0, 2, 1, 3)).reshape(I, N, JM)
    Wr = Wr.reshape(H, IP, N, JM)
    wcr = np.ascontiguousarray(Wr.transpose(1, 2, 0, 3)).reshape(128, H, JM)
    # x rows in the same (i16, n) order per h
    xr = x.reshape(nb, H, IP, N)
    xrows = np.ascontiguousarray(xr.transpose(2, 3, 1, 0)).reshape(128, H, nb)
    # xdg[c, n, b, h] = x[b, h*16+c, n]  (diagonal bands, true bytes only)
    xdg = np.ascontiguousarray(xr.transpose(2, 3, 0, 1))  # [IP, N, nb, H]
    mask = np.zeros((JM, J), np.float32)
    for j in range(J):
        mask[j * M : (j + 1) * M, j] = 1.0
    ident = np.eye(128, dtype=np.float32)
    return (
        wcr.astype(bf),
        xdg.astype(bf),
        xrows.astype(bf),
        mask.astype(bf),
        ident,
    )


def core_in_maps(x, W):
    """Per-core input dicts for run_bass_kernel_spmd."""
    wcr, xdg_all, xt_all, mask, ident = host_prep(x, W)
    in_maps = []
    for c in range(N_CORES):
        bs = slice(c * B_C, (c + 1) * B_C)
        # [IP, N, 32, H] -> [IP, N, NE, E, H]
        xdg_c = np.ascontiguousarray(
            xdg_all[:, :, bs, :].reshape(IP, N, NE, E, H)
        )
        in_maps.append(
            {
                "wcr": wcr,
                "xdg": xdg_c,
                "xt": np.ascontiguousarray(xt_all[:, :, bs]),
                "mask": mask,
                "ident": ident,
            }
        )
    return in_maps


def kernel(x, W):
    x = np.asarray(x, np.float32)
    W = np.asarray(W, np.float32)
    in_maps = core_in_maps(x, W)
    nc = _build_nc()
    res = run_bass_kernel_spmd(nc, in_maps, list(range(N_CORES)))
    out = np.concatenate([res.results[c]["v"] for c in range(N_CORES)], axis=0)
    return out.astype(np.float32)


# revision 57
# speedup vs baseline: 1.2452x; 1.0236x over previous
"""DigitCaps dynamic-routing kernel for Trainium2 (8 NeuronCores, Bass/Tile).

Problem: B=256, IN_CAPS=3200, IN_DIM=8, OUT_CAPS=8, OUT_DIM=16, 3 routing
iterations.  Data-parallel over batch: 32 batches per core.

v2 design (per core):
  - batch processed in 8 "eighths" of 4 batches; per eighth, u_hat is
    created ONCE in SBUF bf16 in BOTH layouts and reused by both routing
    iterations (the baseline recreated it per iteration):
      u_jm [jm=128p, t, b, i]  via K=128 block-diagonal creation matmuls
      u_res [i=128p, t, b, jm] via XBAR DMA-transpose of u_jm tiles
  - x's 16x zero-padded block-diagonal operand is materialized ON CHIP:
    a resident xbz tile is memset to zero once, then only the true x
    bytes are DMA'd into the 16 diagonal bands per eighth (1.6 MB total
    instead of 52 MB of padded streams).
  - a-pass: stationary u_jm tile (128x128 bf16), moving block-diagonal
    v (8/16 cols) -> a^T [i-part, j] so softmax over j is a free-axis
    op on 128 partitions; softmax batched over 8-tile groups.
  - s-pass: stationary c columns (cheap 8-col weight loads), moving
    u_res tiles, accumulating s_ps [J, b, jm] over t with a mask/ones
    extraction (baseline orientation).
  - PSUM->SBUF eviction of created u_hat in 2-tile batches split 2:3
    over DVE / Activation.
  - emission is software-pipelined across eighths so the PE keeps
    working (next eighth's creation) while squash chains run on vector.
"""

import sys

if "/opt/trn_rl_repo" not in sys.path:
    sys.path.insert(0, "/opt/trn_rl_repo")

import ml_dtypes
import numpy as np

import bass_rust
import concourse.bass as bass
import concourse.mybir as mybir
import concourse.tile as tile
from concourse._compat import with_exitstack
from concourse.bass_utils import run_bass_kernel_spmd
from concourse.vector_clock import ScopedClock

# ---------------------------------------------------------------------------
# Walrus on this toolchain rejects multi-wait CTRL instructions;
# TileContext's tail drain aggregates one wait per outstanding semaphore.
# Split the waits across consecutive SP drains.
_TILE_PATCHED = False


def _drain_and_barrier_split(self, tick_clock, wait_clock):
    drain_inst = self.nc.sync.drain()
    wait_clock.add_sem_waits(
        drain_inst.ins, ScopedClock({None: tick_clock.global_clock})
    )
    mi = drain_inst.ins
    waits = list(mi.sync_info.on_wait) if mi.sync_info else []
    if len(waits) > 1:
        si = mi.sync_info
        si.on_wait = waits[:1]
        mi.sync_info = si
        for i in range(1, len(waits)):
            extra = self.nc.sync.drain().ins
            extra.sync_info = bass_rust.SyncInfo(
                on_wait=waits[i : i + 1], on_update=[]
            )
    self.nc.all_engine_barrier()
    assert self.sems is not None
    popped = self.nc._tile_sem_poison_stack.pop()
    assert popped is self._sem_poison
    self.nc.clear_and_free_semaphores(list(self.sems.allocated().values()))
    self.nc.all_engine_barrier()


def _patch_tile():
    global _TILE_PATCHED
    if not _TILE_PATCHED:
        tile.TileContext._drain_and_barrier = _drain_and_barrier_split
        _TILE_PATCHED = True


_SW_COUNT = [0]


def _split_waits(nc):
    """This walrus build allows one sync wait per instruction: hoist extra
    waits onto same-engine NoOp carriers placed just before."""
    for f in nc.m.functions:
        for blk in f.blocks:
            insts = blk.instructions
            if not any(
                inst.sync_info and len(inst.sync_info.on_wait) > 1
                for inst in insts
            ):
                continue
            new = []
            for inst in insts:
                si = inst.sync_info
                waits = list(si.on_wait) if si else []
                if len(waits) > 1:
                    for w in waits[:-1]:
                        _SW_COUNT[0] += 1
                        car = mybir.InstNoOp(
                            name=f"I-sw{_SW_COUNT[0]}", engine=inst.engine
                        )
                        car.sync_info = bass_rust.SyncInfo(
                            on_wait=[w], on_update=[]
                        )
                        new.append(car)
                    si.on_wait = waits[-1:]
                    inst.sync_info = si
                new.append(inst)
            insts[:] = new


# ---------------------------------------------------------------------------
B, I, N, J, M = 256, 3200, 8, 8, 16
JM = J * M  # 128
N_CORES = 8
B_C = B // N_CORES  # 32
T = I // 128  # 25 i-tiles

IP = 16  # i's packed per K-chunk (K = IP*N = 128, uniform row group)
H = I // IP  # 200
CH_T = 128 // IP  # 8 creation chunks per 128-i tile

E = 4  # batches per eighth
NE = B_C // E  # 8 eighths

F32 = mybir.dt.float32
BF16 = mybir.dt.bfloat16


def _squash_chain(nc, small, ps, s_sb, ident, nb):
    """s_sb [128(jm), nb] f32 -> vT [nb, 128] f32.
    squash per capsule j: sq = sum_m s^2, v = sq*s/((1+sq)*sqrt(sq))."""
    sT_ps = ps.tile([nb, JM], F32, tag="sx")
    nc.tensor.matmul(sT_ps[:], s_sb[:], ident[:], is_transpose=True)
    sT = small.tile([nb, J, M], F32, tag="sT")
    nc.vector.tensor_copy(sT[:], sT_ps[:].rearrange("b (j m) -> b j m", m=M))
    s2 = small.tile([nb, J, M], F32, tag="s2")
    nc.vector.tensor_tensor(s2[:], sT[:], sT[:], mybir.AluOpType.mult)
    sq = small.tile([nb, J], F32, tag="sq")
    nc.vector.tensor_reduce(sq[:], s2[:], mybir.AxisListType.X, mybir.AluOpType.add)
    rt = small.tile([nb, J], F32, tag="rt")
    nc.scalar.activation(rt[:], sq[:], mybir.ActivationFunctionType.Sqrt)
    den = small.tile([nb, J], F32, tag="den")
    nc.vector.tensor_scalar_add(den[:], sq[:], 1.0)
    nc.vector.tensor_tensor(den[:], den[:], rt[:], mybir.AluOpType.mult)
    rden = small.tile([nb, J], F32, tag="rden")
    nc.vector.reciprocal(rden[:], den[:])
    scale = small.tile([nb, J], F32, tag="scale")
    nc.vector.tensor_tensor(scale[:], sq[:], rden[:], mybir.AluOpType.mult)
    vT = small.tile([nb, J, M], F32, tag="vT")
    scale_b = bass.AP(
        scale.tensor, scale[:].offset, [scale[:].ap[0], scale[:].ap[1], [0, M]]
    )
    nc.vector.tensor_tensor(vT[:], sT[:], scale_b, mybir.AluOpType.mult)
    return vT


def _vblk_from_vT(nc, small, vT, mask_rep, nb):
    """vT [nb, 128] f32 -> vblk [128(jm), nb, J] bf16 block-diagonal over j.
    Uses the XBAR transpose; only safe for nb >= 32."""
    vT16 = small.tile([nb, JM], BF16, tag="vT16")
    nc.vector.tensor_copy(vT16[:], vT[:])
    vjm = small.tile([JM, nb], BF16, tag="vjm")
    nc.sync.dma_start_transpose(vjm[:], vT16[:])
    vblk = small.tile([JM, nb, J], BF16, tag="vblk_tmp")
    vjm_b = bass.AP(vjm.tensor, vjm[:].offset, [vjm[:].ap[0], vjm[:].ap[1], [0, J]])
    mask_b = bass.AP(
        mask_rep.tensor,
        mask_rep[:].offset,
        [mask_rep[:].ap[0], [0, nb], mask_rep[:].ap[1]],
    )
    nc.vector.tensor_tensor(vblk[:], vjm_b, mask_b, mybir.AluOpType.mult)
    return vblk


@with_exitstack
def build_kernel(ctx, tc, outs, ins, reps=1, stage=3):
    nc = tc.nc
    (v_out,) = outs
    (wcr_d, xblk_d, xt_d, mask_d, ident_d) = ins

    TG = 8  # t-group size for batched softmax

    const = ctx.enter_context(tc.tile_pool(name="const", bufs=1))
    ujmp = ctx.enter_context(tc.tile_pool(name="ujmp", bufs=2))
    uresp = ctx.enter_context(tc.tile_pool(name="uresp", bufs=2))
    sm = ctx.enter_context(tc.tile_pool(name="sm", bufs=2))
    ctp = ctx.enter_context(tc.tile_pool(name="ctp", bufs=8))
    small = ctx.enter_context(tc.tile_pool(name="small", bufs=2))
    psq = ctx.enter_context(tc.tile_pool(name="psq", bufs=1, space="PSUM"))
    cpsp = ctx.enter_context(tc.tile_pool(name="cpsp", bufs=2, space="PSUM"))
    apsp = ctx.enter_context(tc.tile_pool(name="apsp", bufs=2, space="PSUM"))
    saccp = ctx.enter_context(tc.tile_pool(name="saccp", bufs=1, space="PSUM"))

    # Resident constants.  wcr/xt split into chunked DMAs so early matmuls
    # can start before the whole tensor lands.
    wcr = const.tile([128, H, JM], BF16)
    for c4 in range(4):
        nc.sync.dma_start(wcr[:, c4 * 50 : (c4 + 1) * 50, :],
                          wcr_d[:, c4 * 50 : (c4 + 1) * 50, :])
    xt = const.tile([128, H, B_C], BF16)
    nc.scalar.dma_start(xt[:], xt_d[:])
    mask_rep = const.tile([JM, J], BF16)
    nc.sync.dma_start(mask_rep[:], mask_d[:])
    maskT = const.tile([J, JM], BF16)
    nc.sync.dma_start(maskT[:], mask_d[:].rearrange("a b -> b a"))
    ones8 = const.tile([J, 1], BF16)
    nc.vector.memset(ones8[:], 1.0)
    ident = const.tile([128, 128], F32)
    nc.sync.dma_start(ident[:], ident_d[:])

    xsp = ctx.enter_context(tc.tile_pool(name="xsp", bufs=6))
    PF = 6  # prefetch depth: xb tiles DMA'd during the previous routing

    def prefetch(bat):
        tiles = []
        for t in range(PF):
            xb = xsp.tile([128, CH_T, 2 * E, IP], BF16, tag="xb")
            nc.sync.dma_start(xb[:], xblk_d[bat, t])
            tiles.append(xb)
        return tiles

    def create2(bat, pf, ujm0, ures0, ujm1, ures1):
        """Creation for one 8-batch chunk (eighths 2*bat, 2*bat+1): one
        128-col-moving matmul per h halves the LDWEIGHTS-bound matmul
        count; evictions split per eighth across DVE/Act.  All xb stream
        DMAs ride the sync ring and all XBAR transposes the scalar ring:
        mixing them across rings corrupts DMA-completion tracking on HW
        (out-of-order completions on shared semaphore lanes)."""
        for t in range(T):
            if t < PF:
                xb = pf[t]
            else:
                xb = xsp.tile([128, CH_T, 2 * E, IP], BF16, tag="xb")
                nc.sync.dma_start(xb[:], xblk_d[bat, t])
            cps = cpsp.tile([JM, CH_T, 2 * E, IP], F32, tag="cps")
            for g in range(CH_T):
                h = t * CH_T + g
                nc.tensor.matmul(
                    cps[:, g, :, :], wcr[:, h, :], xb[:, g, :, :],
                    start=True, stop=True,
                )
            for half, ujm in ((0, ujm0), (1, ujm1)):
                dst = ujm[:, t, :, :].rearrange("p b (g i) -> p g b i", i=IP)
                src = cps[:, :, half * E : (half + 1) * E, :]
                if (t + half) % 2 == 0:
                    nc.vector.tensor_copy(dst, src)
                else:
                    nc.scalar.activation(
                        dst, src, mybir.ActivationFunctionType.Copy
                    )
            if t % 5 == 4:
                for half, (ujm, ures) in enumerate(
                    ((ujm0, ures0), (ujm1, ures1))
                ):
                    nc.scalar.dma_start_transpose(
                        ures[:, t - 4 : t + 1, :, :],
                        ujm[:, t - 4 : t + 1, :, :],
                    )

    def apass_softmax(q, it, ujm, vblk):
        """a-pass + batched softmax for eighth q, iteration it (2|3).
        Returns list of c_t tiles [128(i), TG, E, J] bf16 per t-group."""
        nslot = it - 1
        cts = []
        for g0 in range(0, T, TG):
            g1 = min(g0 + TG, T)
            ng = g1 - g0
            aps = apsp.tile([128, TG, E, 16], F32, tag="aps")
            for t in range(g0, g1):
                for b in range(E):
                    nc.tensor.matmul(
                        aps[:, t - g0, b, : nslot * J],
                        ujm[:, t, b, :],
                        vblk[:, q * E + b, :nslot, :],
                        start=True, stop=True,
                    )
            av = aps[:, :ng]
            e = sm.tile([128, TG, E, J], BF16, tag="e")
            if it == 2:
                # exp straight from PSUM; no logits copy needed
                nc.scalar.activation(
                    e[:, :ng], av[:, :, :, 0:J],
                    mybir.ActivationFunctionType.Exp,
                )
            else:
                lg0 = sm.tile([128, TG, E, J], F32, tag="lg0")
                nc.scalar.activation(
                    lg0[:, :ng], av[:, :, :, 0:J],
                    mybir.ActivationFunctionType.Copy,
                )
                lg = sm.tile([128, TG, E, J], F32, tag="lg")
                nc.vector.tensor_tensor(
                    lg[:, :ng], lg0[:, :ng], av[:, :, :, J : 2 * J],
                    mybir.AluOpType.add,
                )
                nc.scalar.activation(
                    e[:, :ng], lg[:, :ng], mybir.ActivationFunctionType.Exp
                )
            z = sm.tile([128, TG, E], F32, tag="z")
            nc.vector.tensor_reduce(
                z[:, :ng], e[:, :ng], mybir.AxisListType.X, mybir.AluOpType.add
            )
            rz = sm.tile([128, TG, E], F32, tag="rz")
            nc.vector.reciprocal(rz[:, :ng], z[:, :ng])
            c_t = ctp.tile([128, TG, E, J], BF16, tag="c_t")
            rzb = bass.AP(
                rz.tensor, rz[:, :ng].offset,
                [rz[:].ap[0], [rz[:].ap[1][0], ng], rz[:].ap[2], [0, J]],
            )
            nc.vector.tensor_tensor(
                c_t[:, :ng], e[:, :ng], rzb, mybir.AluOpType.mult
            )
            cts.append(c_t)
        return cts

    def spass(q, it, ures, cts):
        """s-pass: stationary c columns (cheap weight loads), moving u_res
        tiles; accumulates s_ps [J, E, JM] over t.  Returns s_sb [jm, E]."""
        sacc = saccp.tile([J, E, JM], F32, tag="sacc")
        nc.vector.memset(sacc[:], 0.0)
        for g0 in range(0, T, TG):
            c_t = cts[g0 // TG]
            for t in range(g0, min(g0 + TG, T)):
                for b in range(E):
                    nc.tensor.matmul(
                        sacc[:, b, :],
                        c_t[:, t - g0, b, :],
                        ures[:, t, b, :],
                        start=False, stop=False, skip_group_check=True,
                    )
        msb = small.tile([J, E, JM], BF16, tag="msb")
        maskT_b = bass.AP(
            maskT.tensor, maskT[:].offset,
            [maskT[:].ap[0], [0, E], maskT[:].ap[1]],
        )
        nc.vector.tensor_tensor(msb[:], sacc[:], maskT_b, mybir.AluOpType.mult)
        s2_ps = psq.tile([JM, E], F32, tag="sx")
        for b in range(E):
            nc.tensor.matmul(
                s2_ps[:, b : b + 1], msb[:, b, :], ones8[:],
                start=True, stop=True,
            )
        s_sb = small.tile([JM, E], F32, tag="s_sb")
        nc.vector.tensor_copy(s_sb[:], s2_ps[:])
        return s_sb

    def vblk_write(q, vTh, vblk):
        """vTh [E, 128] f32 -> vblk[:, q*E:(q+1)*E, 1, :] via PE transpose."""
        vps = psq.tile([JM, E], F32, tag="sx")
        nc.tensor.matmul(
            vps[:], vTh[:], ident[0:E, 0:E], is_transpose=True
        )
        vjm = small.tile([JM, E], BF16, tag="vjms")
        nc.vector.tensor_copy(vjm[:], vps[:])
        vjm_b = bass.AP(
            vjm.tensor, vjm[:].offset, [vjm[:].ap[0], vjm[:].ap[1], [0, J]]
        )
        mask_b = bass.AP(
            mask_rep.tensor, mask_rep[:].offset,
            [mask_rep[:].ap[0], [0, E], mask_rep[:].ap[1]],
        )
        nc.vector.tensor_tensor(
            vblk[:, q * E : (q + 1) * E, 1, :], vjm_b, mask_b,
            mybir.AluOpType.mult,
        )

    vblk = const.tile([JM, B_C, 2, J], BF16, tag="vblk")

    for rep in range(reps):
        # ---- iteration 1 (all batches): s1 = (1/8) sum_(i,n) W x ----------
        s1_ps = psq.tile([JM, B_C], F32, tag="sx")
        for h in range(H):
            nc.tensor.matmul(
                s1_ps[:], wcr[:, h, :], xt[:, h, :],
                start=(h == 0), stop=(h == H - 1),
            )
        s_sb = small.tile([JM, B_C], F32, tag="s_all")
        nc.vector.tensor_scalar_mul(s_sb[:], s1_ps[:], 1.0 / J)
        vT = _squash_chain(nc, small, psq, s_sb, ident, B_C)
        vb = _vblk_from_vT(nc, small, vT, mask_rep, B_C)
        nc.vector.tensor_copy(vblk[:, :, 0, :], vb[:])

        ujm = [None] * NE
        ures = [None] * NE

        def mk_pair(bat, pf):
            for q in (2 * bat, 2 * bat + 1):
                uj = ujmp.tile([JM, T, E, 128], BF16, tag="ujm")
                ur = uresp.tile([128, T, E, JM], BF16, tag="ures")
                ujm[q] = uj
                ures[q] = ur
            create2(bat, pf, ujm[2 * bat], ures[2 * bat],
                    ujm[2 * bat + 1], ures[2 * bat + 1])

        mk_pair(0, prefetch(0))
        if stage < 3:
            # ablation: creation only
            for bat in range(1, NE // 2):
                mk_pair(bat, prefetch(bat))
            nc.sync.dma_start(
                v_out[:].rearrange("b j m -> b (j m)")[:, :], vT[:]
            )
            continue

        cts2 = apass_softmax(0, 2, ujm[0], vblk)
        pf = None
        for q in range(NE):
            if q % 2 == 0 and q + 2 < NE:
                # stream the next pair's first xb tiles during this routing
                pf = prefetch(q // 2 + 1)
            # it2 finish: s-pass, squash, vblk slot 1
            s_sb2 = spass(q, 2, ures[q], cts2)
            vT2 = _squash_chain(nc, small, psq, s_sb2, ident, E)
            vblk_write(q, vT2, vblk)
            cts3 = apass_softmax(q, 3, ujm[q], vblk)
            if q % 2 == 0:
                cts2 = apass_softmax(q + 1, 2, ujm[q + 1], vblk)
            s_sb3 = spass(q, 3, ures[q], cts3)
            vT3 = _squash_chain(nc, small, psq, s_sb3, ident, E)
            nc.sync.dma_start(
                v_out[:].rearrange("b j m -> b (j m)")[q * E : (q + 1) * E, :],
                vT3[:],
            )
            if q % 2 == 1 and q + 1 < NE:
                # next pair's creation; all readers of this pair's u done
                mk_pair((q + 1) // 2, pf)
                cts2 = apass_softmax(q + 1, 2, ujm[q + 1], vblk)


_NC_CACHE = {}


def _build_nc(reps=1, stage=3):
    key = (reps, stage)
    if key not in _NC_CACHE:
        _patch_tile()
        nc = bass.Bass("TRN2", target_bir_lowering=False, debug=False)
        wcr_d = nc.dram_tensor("wcr", [128, H, JM], BF16, kind="ExternalInput").ap()
        xblk_d = nc.dram_tensor(
            "xblk", [NE // 2, T, 128, CH_T, 2 * E, IP], BF16,
            kind="ExternalInput",
        ).ap()
        xt_d = nc.dram_tensor("xt", [128, H, B_C], BF16, kind="ExternalInput").ap()
        mask_d = nc.dram_tensor("mask", [JM, J], BF16, kind="ExternalInput").ap()
        ident_d = nc.dram_tensor("ident", [128, 128], F32, kind="ExternalInput").ap()
        v_d = nc.dram_tensor("v", [B_C, J, M], F32, kind="ExternalOutput").ap()
        with tile.TileContext(nc) as tc:
            build_kernel(
                tc,
                [v_d],
                [wcr_d, xblk_d, xt_d, mask_d, ident_d],
                reps=reps,
                stage=stage,
            )
        _split_waits(nc)
        _NC_CACHE[key] = nc
    return _NC_CACHE[key]


def host_prep(x, W):
    """Returns (wcr, xdg_all, xt_all, mask, ident); x-deriveds cover all B.
    Row order of the 128 K-rows is (i16, n): i = h*IP + i16."""
    bf = ml_dtypes.bfloat16
    nb = x.shape[0]
    # wcr[(i16*N + n), h, jm] = W[h*IP + i16, j, n, m]
    Wr = np.ascontiguousarray(W.transpose(0, 2, 1, 3)).reshape(I, N, JM)
    Wr = Wr.reshape(H, IP, N, JM)
    wcr = np.ascontiguousarray(Wr.transpose(1, 2, 0, 3)).reshape(128, H, JM)
    # x rows in the same (i16, n) order per h
    xr = x.reshape(nb, H, IP, N)
    xrows = np.ascontiguousarray(xr.transpose(2, 3, 1, 0)).reshape(128, H, nb)
    # zero-padded block-diagonal x operand rows
    rows = np.arange(128)
    i16_of_row = rows // N
    xblk = np.zeros((128, H, nb, IP), np.float32)
    for r in range(128):
        xblk[r, :, :, i16_of_row[r]] = xrows[r]
    mask = np.zeros((JM, J), np.float32)
    for j in range(J):
        mask[j * M : (j + 1) * M, j] = 1.0
    ident = np.eye(128, dtype=np.float32)
    return (
        wcr.astype(bf),
        xblk.astype(bf),
        xrows.astype(bf),
        mask.astype(bf),
        ident,
    )


def regroup(xblk_core):
    """xblk [128,H,nb,IP] -> [nb//8, T, 128, CH_T, 8, IP]."""
    nb = xblk_core.shape[2]
    xb = xblk_core.reshape(128, T, CH_T, nb, IP)
    xb = xb.transpose(3, 1, 0, 2, 4)  # [nb, T, 128, CH_T, IP]
    xb = xb.reshape(nb // 8, 8, T, 128, CH_T, IP).transpose(0, 2, 3, 4, 1, 5)
    return np.ascontiguousarray(xb)


def core_in_maps(x, W):
    """Per-core input dicts for run_bass_kernel_spmd."""
    wcr, xblk_all, xt_all, mask, ident = host_prep(x, W)
    in_maps = []
    for c in range(N_CORES):
        bs = slice(c * B_C, (c + 1) * B_C)
        in_maps.append(
            {
                "wcr": wcr,
                "xblk": regroup(xblk_all[:, :, bs, :]),
                "xt": np.ascontiguousarray(xt_all[:, :, bs]),
                "mask": mask,
                "ident": ident,
            }
        )
    return in_maps


def kernel(x, W):
    x = np.asarray(x, np.float32)
    W = np.asarray(W, np.float32)
    in_maps = core_in_maps(x, W)
    nc = _build_nc()
    res = run_bass_kernel_spmd(nc, in_maps, list(range(N_CORES)))
    out = np.concatenate([res.results[c]["v"] for c in range(N_CORES)], axis=0)
    return out.astype(np.float32)


# revision 61
# speedup vs baseline: 1.4848x; 1.1924x over previous
"""DigitCaps dynamic-routing kernel for Trainium2 (8 NeuronCores, Bass/Tile).

Problem: B=256, IN_CAPS=3200, IN_DIM=8, OUT_CAPS=8, OUT_DIM=16, 3 routing
iterations.  Data-parallel over batch: 32 batches per core.

v2 design (per core):
  - batch processed in 8 "eighths" of 4 batches; per eighth, u_hat is
    created ONCE in SBUF bf16 in BOTH layouts and reused by both routing
    iterations (the baseline recreated it per iteration):
      u_jm [jm=128p, t, b, i]  via K=128 block-diagonal creation matmuls
      u_res [i=128p, t, b, jm] via XBAR DMA-transpose of u_jm tiles
  - creation runs per PAIR of eighths (8 batches): one 128-col-moving
    matmul per K-chunk halves the LDWEIGHTS-bound matmul count; the
    zero-padded x operand streams from DRAM on the sync ring only, with
    a 6-tile prefetch issued during the previous pair's routing.  XBAR
    transposes ride the scalar ring exclusively: mixing stream DMAs and
    XBARs across rings corrupts DMA-completion tracking on HW.
  - a-pass: stationary u_jm tile (128x128 bf16), moving block-diagonal
    v (8/16 cols) -> a^T [i-part, j] so softmax over j is a free-axis
    op on 128 partitions; softmax batched over 8-tile groups.
  - s-pass: stationary c columns (cheap 8-col weight loads), moving
    u_res tiles, accumulating s_ps [J, b, jm] over t with a mask/ones
    extraction (baseline orientation).
  - PSUM->SBUF eviction of created u_hat in 2-tile batches split 2:3
    over DVE / Activation.
  - emission is software-pipelined across eighths so the PE keeps
    working (next eighth's creation) while squash chains run on vector.
"""

import sys

if "/opt/trn_rl_repo" not in sys.path:
    sys.path.insert(0, "/opt/trn_rl_repo")

import ml_dtypes
import numpy as np

import bass_rust
import concourse.bass as bass
import concourse.mybir as mybir
import concourse.tile as tile
from concourse._compat import with_exitstack
from concourse.bass_utils import run_bass_kernel_spmd
from concourse.vector_clock import ScopedClock

# ---------------------------------------------------------------------------
# Walrus on this toolchain rejects multi-wait CTRL instructions;
# TileContext's tail drain aggregates one wait per outstanding semaphore.
# Split the waits across consecutive SP drains.
_TILE_PATCHED = False


def _drain_and_barrier_split(self, tick_clock, wait_clock):
    drain_inst = self.nc.sync.drain()
    wait_clock.add_sem_waits(
        drain_inst.ins, ScopedClock({None: tick_clock.global_clock})
    )
    mi = drain_inst.ins
    waits = list(mi.sync_info.on_wait) if mi.sync_info else []
    if len(waits) > 1:
        si = mi.sync_info
        si.on_wait = waits[:1]
        mi.sync_info = si
        for i in range(1, len(waits)):
            extra = self.nc.sync.drain().ins
            extra.sync_info = bass_rust.SyncInfo(
                on_wait=waits[i : i + 1], on_update=[]
            )
    self.nc.all_engine_barrier()
    assert self.sems is not None
    popped = self.nc._tile_sem_poison_stack.pop()
    assert popped is self._sem_poison
    self.nc.clear_and_free_semaphores(list(self.sems.allocated().values()))
    self.nc.all_engine_barrier()


def _patch_tile():
    global _TILE_PATCHED
    if not _TILE_PATCHED:
        tile.TileContext._drain_and_barrier = _drain_and_barrier_split
        _TILE_PATCHED = True


_SW_COUNT = [0]


def _split_waits(nc):
    """This walrus build allows one sync wait per instruction: hoist extra
    waits onto same-engine NoOp carriers placed just before."""
    for f in nc.m.functions:
        for blk in f.blocks:
            insts = blk.instructions
            if not any(
                inst.sync_info and len(inst.sync_info.on_wait) > 1
                for inst in insts
            ):
                continue
            new = []
            for inst in insts:
                si = inst.sync_info
                waits = list(si.on_wait) if si else []
                if len(waits) > 1:
                    for w in waits[:-1]:
                        _SW_COUNT[0] += 1
                        car = mybir.InstNoOp(
                            name=f"I-sw{_SW_COUNT[0]}", engine=inst.engine
                        )
                        car.sync_info = bass_rust.SyncInfo(
                            on_wait=[w], on_update=[]
                        )
                        new.append(car)
                    si.on_wait = waits[-1:]
                    inst.sync_info = si
                new.append(inst)
            insts[:] = new


# ---------------------------------------------------------------------------
B, I, N, J, M = 256, 3200, 8, 8, 16
JM = J * M  # 128
N_CORES = 8
B_C = B // N_CORES  # 32
T = I // 128  # 25 i-tiles

IP = 16  # i's packed per K-chunk (K = IP*N = 128, uniform row group)
H = I // IP  # 200
CH_T = 128 // IP  # 8 creation chunks per 128-i tile

E = 4  # batches per eighth
NE = B_C // E  # 8 eighths

F32 = mybir.dt.float32
BF16 = mybir.dt.bfloat16


def _squash_chain(nc, small, ps, s_sb, ident, nb):
    """s_sb [128(jm), nb] f32 -> vT [nb, 128] f32.
    squash per capsule j: sq = sum_m s^2, v = sq*s/((1+sq)*sqrt(sq))."""
    sT_ps = ps.tile([nb, JM], F32, tag="sx")
    nc.tensor.matmul(sT_ps[:], s_sb[:], ident[:], is_transpose=True)
    sT = small.tile([nb, J, M], F32, tag="sT")
    nc.vector.tensor_copy(sT[:], sT_ps[:].rearrange("b (j m) -> b j m", m=M))
    s2 = small.tile([nb, J, M], F32, tag="s2")
    nc.vector.tensor_tensor(s2[:], sT[:], sT[:], mybir.AluOpType.mult)
    sq = small.tile([nb, J], F32, tag="sq")
    nc.vector.tensor_reduce(sq[:], s2[:], mybir.AxisListType.X, mybir.AluOpType.add)
    rt = small.tile([nb, J], F32, tag="rt")
    nc.scalar.activation(rt[:], sq[:], mybir.ActivationFunctionType.Sqrt)
    den = small.tile([nb, J], F32, tag="den")
    nc.vector.tensor_scalar_add(den[:], sq[:], 1.0)
    nc.vector.tensor_tensor(den[:], den[:], rt[:], mybir.AluOpType.mult)
    rden = small.tile([nb, J], F32, tag="rden")
    nc.vector.reciprocal(rden[:], den[:])
    scale = small.tile([nb, J], F32, tag="scale")
    nc.vector.tensor_tensor(scale[:], sq[:], rden[:], mybir.AluOpType.mult)
    vT = small.tile([nb, J, M], F32, tag="vT")
    scale_b = bass.AP(
        scale.tensor, scale[:].offset, [scale[:].ap[0], scale[:].ap[1], [0, M]]
    )
    nc.vector.tensor_tensor(vT[:], sT[:], scale_b, mybir.AluOpType.mult)
    return vT


def _vblk_from_vT(nc, small, vT, mask_rep, nb):
    """vT [nb, 128] f32 -> vblk [128(jm), nb, J] bf16 block-diagonal over j.
    Uses the XBAR transpose; only safe for nb >= 32."""
    vT16 = small.tile([nb, JM], BF16, tag="vT16")
    nc.vector.tensor_copy(vT16[:], vT[:])
    vjm = small.tile([JM, nb], BF16, tag="vjm")
    nc.sync.dma_start_transpose(vjm[:], vT16[:])
    vblk = small.tile([JM, nb, J], BF16, tag="vblk_tmp")
    vjm_b = bass.AP(vjm.tensor, vjm[:].offset, [vjm[:].ap[0], vjm[:].ap[1], [0, J]])
    mask_b = bass.AP(
        mask_rep.tensor,
        mask_rep[:].offset,
        [mask_rep[:].ap[0], [0, nb], mask_rep[:].ap[1]],
    )
    nc.vector.tensor_tensor(vblk[:], vjm_b, mask_b, mybir.AluOpType.mult)
    return vblk


@with_exitstack
def build_kernel(ctx, tc, outs, ins, reps=1, stage=3):
    nc = tc.nc
    (v_out,) = outs
    (wcr_d, xblk_d, xt_d, mask_d, ident_d) = ins

    TG = 8  # t-group size for batched softmax

    const = ctx.enter_context(tc.tile_pool(name="const", bufs=1))
    ujmp = ctx.enter_context(tc.tile_pool(name="ujmp", bufs=2))
    uresp = ctx.enter_context(tc.tile_pool(name="uresp", bufs=2))
    sm = ctx.enter_context(tc.tile_pool(name="sm", bufs=2))
    ctp = ctx.enter_context(tc.tile_pool(name="ctp", bufs=8))
    small = ctx.enter_context(tc.tile_pool(name="small", bufs=2))
    psq = ctx.enter_context(tc.tile_pool(name="psq", bufs=1, space="PSUM"))
    cpsp = ctx.enter_context(tc.tile_pool(name="cpsp", bufs=2, space="PSUM"))
    apsp = ctx.enter_context(tc.tile_pool(name="apsp", bufs=2, space="PSUM"))
    saccp = ctx.enter_context(tc.tile_pool(name="saccp", bufs=1, space="PSUM"))

    # Resident constants.  wcr/xt split into chunked DMAs so early matmuls
    # can start before the whole tensor lands.
    wcr = const.tile([128, H, JM], BF16)
    for c4 in range(4):
        nc.sync.dma_start(wcr[:, c4 * 50 : (c4 + 1) * 50, :],
                          wcr_d[:, c4 * 50 : (c4 + 1) * 50, :])
    xt = const.tile([128, H, B_C], BF16)
    nc.scalar.dma_start(xt[:], xt_d[:])
    mask_rep = const.tile([JM, J], BF16)
    nc.sync.dma_start(mask_rep[:], mask_d[:])
    maskT = const.tile([J, JM], BF16)
    nc.sync.dma_start(maskT[:], mask_d[:].rearrange("a b -> b a"))
    ones8 = const.tile([J, 1], BF16)
    nc.vector.memset(ones8[:], 1.0)
    ident = const.tile([128, 128], F32)
    nc.sync.dma_start(ident[:], ident_d[:])

    xsp = ctx.enter_context(tc.tile_pool(name="xsp", bufs=6))
    PF = 6  # prefetch depth: xb tiles DMA'd during the previous routing

    def prefetch(bat):
        tiles = []
        for t in range(PF):
            xb = xsp.tile([128, CH_T, 2 * E, IP], BF16, tag="xb")
            nc.sync.dma_start(xb[:], xblk_d[bat, t])
            tiles.append(xb)
        return tiles

    def create2(bat, pf, ujm0, ures0, ujm1, ures1, s1_ps=None):
        """Creation for one 8-batch chunk (eighths 2*bat, 2*bat+1): one
        128-col-moving matmul per h halves the LDWEIGHTS-bound matmul
        count; evictions split per eighth across DVE/Act.  All xb stream
        DMAs ride the sync ring and all XBAR transposes the scalar ring:
        mixing them across rings corrupts DMA-completion tracking on HW
        (out-of-order completions on shared semaphore lanes)."""
        for t in range(T):
            if t < PF:
                xb = pf[t]
            else:
                xb = xsp.tile([128, CH_T, 2 * E, IP], BF16, tag="xb")
                nc.sync.dma_start(xb[:], xblk_d[bat, t])
            cps = cpsp.tile([JM, CH_T, 2 * E, IP], F32, tag="cps")
            for g in range(CH_T):
                h = t * CH_T + g
                nc.tensor.matmul(
                    cps[:, g, :, :], wcr[:, h, :], xb[:, g, :, :],
                    start=True, stop=True,
                )
                if s1_ps is not None:
                    # iteration-1 partial with the SAME stationary wcr[:, h]
                    # as the adjacent creation matmul (weight-load reuse)
                    nc.tensor.matmul(
                        s1_ps[:], wcr[:, h, :], xt[:, h, :],
                        start=False, stop=False, skip_group_check=True,
                    )
            for half, ujm in ((0, ujm0), (1, ujm1)):
                dst = ujm[:, t, :, :].rearrange("p b (g i) -> p g b i", i=IP)
                src = cps[:, :, half * E : (half + 1) * E, :]
                if (t + half) % 2 == 0:
                    nc.vector.tensor_copy(dst, src)
                else:
                    nc.scalar.activation(
                        dst, src, mybir.ActivationFunctionType.Copy
                    )
            if t % 5 == 4:
                for half, (ujm, ures) in enumerate(
                    ((ujm0, ures0), (ujm1, ures1))
                ):
                    nc.scalar.dma_start_transpose(
                        ures[:, t - 4 : t + 1, :, :],
                        ujm[:, t - 4 : t + 1, :, :],
                    )

    def apass_softmax(q, it, ujm, vblk):
        """a-pass + batched softmax for eighth q, iteration it (2|3).
        Returns list of c_t tiles [128(i), TG, E, J] bf16 per t-group."""
        nslot = it - 1
        cts = []
        for g0 in range(0, T, TG):
            g1 = min(g0 + TG, T)
            ng = g1 - g0
            aps = apsp.tile([128, TG, E, 16], F32, tag="aps")
            for t in range(g0, g1):
                for b in range(E):
                    nc.tensor.matmul(
                        aps[:, t - g0, b, : nslot * J],
                        ujm[:, t, b, :],
                        vblk[:, q * E + b, :nslot, :],
                        start=True, stop=True,
                    )
            av = aps[:, :ng]
            e = sm.tile([128, TG, E, J], BF16, tag="e")
            if it == 2:
                # exp straight from PSUM; no logits copy needed
                nc.scalar.activation(
                    e[:, :ng], av[:, :, :, 0:J],
                    mybir.ActivationFunctionType.Exp,
                )
            else:
                lg0 = sm.tile([128, TG, E, J], F32, tag="lg0")
                nc.scalar.activation(
                    lg0[:, :ng], av[:, :, :, 0:J],
                    mybir.ActivationFunctionType.Copy,
                )
                lg = sm.tile([128, TG, E, J], F32, tag="lg")
                nc.vector.tensor_tensor(
                    lg[:, :ng], lg0[:, :ng], av[:, :, :, J : 2 * J],
                    mybir.AluOpType.add,
                )
                nc.scalar.activation(
                    e[:, :ng], lg[:, :ng], mybir.ActivationFunctionType.Exp
                )
            z = sm.tile([128, TG, E], F32, tag="z")
            nc.vector.tensor_reduce(
                z[:, :ng], e[:, :ng], mybir.AxisListType.X, mybir.AluOpType.add
            )
            rz = sm.tile([128, TG, E], F32, tag="rz")
            nc.vector.reciprocal(rz[:, :ng], z[:, :ng])
            c_t = ctp.tile([128, TG, E, J], BF16, tag="c_t")
            rzb = bass.AP(
                rz.tensor, rz[:, :ng].offset,
                [rz[:].ap[0], [rz[:].ap[1][0], ng], rz[:].ap[2], [0, J]],
            )
            nc.vector.tensor_tensor(
                c_t[:, :ng], e[:, :ng], rzb, mybir.AluOpType.mult
            )
            cts.append(c_t)
        return cts

    def spass(q, it, ures, cts):
        """s-pass: stationary c columns (cheap weight loads), moving u_res
        tiles; accumulates s_ps [J, E, JM] over t.  Returns s_sb [jm, E]."""
        sacc = saccp.tile([J, E, JM], F32, tag="sacc")
        nc.vector.memset(sacc[:], 0.0)
        for g0 in range(0, T, TG):
            c_t = cts[g0 // TG]
            for t in range(g0, min(g0 + TG, T)):
                for b in range(E):
                    nc.tensor.matmul(
                        sacc[:, b, :],
                        c_t[:, t - g0, b, :],
                        ures[:, t, b, :],
                        start=False, stop=False, skip_group_check=True,
                    )
        msb = small.tile([J, E, JM], BF16, tag="msb")
        maskT_b = bass.AP(
            maskT.tensor, maskT[:].offset,
            [maskT[:].ap[0], [0, E], maskT[:].ap[1]],
        )
        nc.vector.tensor_tensor(msb[:], sacc[:], maskT_b, mybir.AluOpType.mult)
        s2_ps = psq.tile([JM, E], F32, tag="sx")
        for b in range(E):
            nc.tensor.matmul(
                s2_ps[:, b : b + 1], msb[:, b, :], ones8[:],
                start=True, stop=True,
            )
        s_sb = small.tile([JM, E], F32, tag="s_sb")
        nc.vector.tensor_copy(s_sb[:], s2_ps[:])
        return s_sb

    def vblk_write(q, vTh, vblk):
        """vTh [E, 128] f32 -> vblk[:, q*E:(q+1)*E, 1, :] via PE transpose."""
        vps = psq.tile([JM, E], F32, tag="sx")
        nc.tensor.matmul(
            vps[:], vTh[:], ident[0:E, 0:E], is_transpose=True
        )
        vjm = small.tile([JM, E], BF16, tag="vjms")
        nc.vector.tensor_copy(vjm[:], vps[:])
        vjm_b = bass.AP(
            vjm.tensor, vjm[:].offset, [vjm[:].ap[0], vjm[:].ap[1], [0, J]]
        )
        mask_b = bass.AP(
            mask_rep.tensor, mask_rep[:].offset,
            [mask_rep[:].ap[0], [0, E], mask_rep[:].ap[1]],
        )
        nc.vector.tensor_tensor(
            vblk[:, q * E : (q + 1) * E, 1, :], vjm_b, mask_b,
            mybir.AluOpType.mult,
        )

    vblk = const.tile([JM, B_C, 2, J], BF16, tag="vblk")

    for rep in range(reps):
        # iteration-1 accumulator; filled during pair-0 creation (shared
        # wcr stationaries)
        s1_ps = psq.tile([JM, B_C], F32, tag="sx")
        nc.vector.memset(s1_ps[:], 0.0)

        ujm = [None] * NE
        ures = [None] * NE

        def mk_pair(bat, pf, s1=None):
            for q in (2 * bat, 2 * bat + 1):
                uj = ujmp.tile([JM, T, E, 128], BF16, tag="ujm")
                ur = uresp.tile([128, T, E, JM], BF16, tag="ures")
                ujm[q] = uj
                ures[q] = ur
            create2(bat, pf, ujm[2 * bat], ures[2 * bat],
                    ujm[2 * bat + 1], ures[2 * bat + 1], s1_ps=s1)

        mk_pair(0, prefetch(0), s1_ps)
        # ---- iteration 1 (all batches): s1 = (1/8) sum_(i,n) W x ----------
        s_sb = small.tile([JM, B_C], F32, tag="s_all")
        nc.vector.tensor_scalar_mul(s_sb[:], s1_ps[:], 1.0 / J)
        vT = _squash_chain(nc, small, psq, s_sb, ident, B_C)
        vb = _vblk_from_vT(nc, small, vT, mask_rep, B_C)
        nc.vector.tensor_copy(vblk[:, :, 0, :], vb[:])
        if stage < 3:
            # ablation: creation only
            for bat in range(1, NE // 2):
                mk_pair(bat, prefetch(bat))
            nc.sync.dma_start(
                v_out[:].rearrange("b j m -> b (j m)")[:, :], vT[:]
            )
            continue

        cts2 = apass_softmax(0, 2, ujm[0], vblk)
        pf = None
        for q in range(NE):
            if q % 2 == 0 and q + 2 < NE:
                # stream the next pair's first xb tiles during this routing
                pf = prefetch(q // 2 + 1)
            # it2 finish: s-pass, squash, vblk slot 1
            s_sb2 = spass(q, 2, ures[q], cts2)
            vT2 = _squash_chain(nc, small, psq, s_sb2, ident, E)
            vblk_write(q, vT2, vblk)
            cts3 = apass_softmax(q, 3, ujm[q], vblk)
            if q % 2 == 0:
                cts2 = apass_softmax(q + 1, 2, ujm[q + 1], vblk)
            s_sb3 = spass(q, 3, ures[q], cts3)
            vT3 = _squash_chain(nc, small, psq, s_sb3, ident, E)
            nc.sync.dma_start(
                v_out[:].rearrange("b j m -> b (j m)")[q * E : (q + 1) * E, :],
                vT3[:],
            )
            if q % 2 == 1 and q + 1 < NE:
                # next pair's creation; all readers of this pair's u done
                mk_pair((q + 1) // 2, pf)
                cts2 = apass_softmax(q + 1, 2, ujm[q + 1], vblk)


_NC_CACHE = {}


def _build_nc(reps=1, stage=3):
    key = (reps, stage)
    if key not in _NC_CACHE:
        _patch_tile()
        nc = bass.Bass("TRN2", target_bir_lowering=False, debug=False)
        wcr_d = nc.dram_tensor("wcr", [128, H, JM], BF16, kind="ExternalInput").ap()
        xblk_d = nc.dram_tensor(
            "xblk", [NE // 2, T, 128, CH_T, 2 * E, IP], BF16,
            kind="ExternalInput",
        ).ap()
        xt_d = nc.dram_tensor("xt", [128, H, B_C], BF16, kind="ExternalInput").ap()
        mask_d = nc.dram_tensor("mask", [JM, J], BF16, kind="ExternalInput").ap()
        ident_d = nc.dram_tensor("ident", [128, 128], F32, kind="ExternalInput").ap()
        v_d = nc.dram_tensor("v", [B_C, J, M], F32, kind="ExternalOutput").ap()
        with tile.TileContext(nc) as tc:
            build_kernel(
                tc,
                [v_d],
                [wcr_d, xblk_d, xt_d, mask_d, ident_d],
                reps=reps,
                stage=stage,
            )
        _split_waits(nc)
        _NC_CACHE[key] = nc
    return _NC_CACHE[key]


def host_prep(x, W):
    """Returns (wcr, xdg_all, xt_all, mask, ident); x-deriveds cover all B.
    Row order of the 128 K-rows is (i16, n): i = h*IP + i16."""
    bf = ml_dtypes.bfloat16
    nb = x.shape[0]
    # wcr[(i16*N + n), h, jm] = W[h*IP + i16, j, n, m]
    Wr = np.ascontiguousarray(W.transpose(0, 2, 1, 3)).reshape(I, N, JM)
    Wr = Wr.reshape(H, IP, N, JM)
    wcr = np.ascontiguousarray(Wr.transpose(1, 2, 0, 3)).reshape(128, H, JM)
    # x rows in the same (i16, n) order per h
    xr = x.reshape(nb, H, IP, N)
    xrows = np.ascontiguousarray(xr.transpose(2, 3, 1, 0)).reshape(128, H, nb)
    # zero-padded block-diagonal x operand rows
    rows = np.arange(128)
    i16_of_row = rows // N
    xblk = np.zeros((128, H, nb, IP), np.float32)
    for r in range(128):
        xblk[r, :, :, i16_of_row[r]] = xrows[r]
    mask = np.zeros((JM, J), np.float32)
    for j in range(J):
        mask[j * M : (j + 1) * M, j] = 1.0
    ident = np.eye(128, dtype=np.float32)
    return (
        wcr.astype(bf),
        xblk.astype(bf),
        xrows.astype(bf),
        mask.astype(bf),
        ident,
    )


def regroup(xblk_core):
    """xblk [128,H,nb,IP] -> [nb//8, T, 128, CH_T, 8, IP]."""
    nb = xblk_core.shape[2]
    xb = xblk_core.reshape(128, T, CH_T, nb, IP)
    xb = xb.transpose(3, 1, 0, 2, 4)  # [nb, T, 128, CH_T, IP]
    xb = xb.reshape(nb // 8, 8, T, 128, CH_T, IP).transpose(0, 2, 3, 4, 1, 5)
    return np.ascontiguousarray(xb)


def core_in_maps(x, W):
    """Per-core input dicts for run_bass_kernel_spmd."""
    wcr, xblk_all, xt_all, mask, ident = host_prep(x, W)
    in_maps = []
    for c in range(N_CORES):
        bs = slice(c * B_C, (c + 1) * B_C)
        in_maps.append(
            {
                "wcr": wcr,
                "xblk": regroup(xblk_all[:, :, bs, :]),
                "xt": np.ascontiguousarray(xt_all[:, :, bs]),
                "mask": mask,
                "ident": ident,
            }
        )
    return in_maps


def kernel(x, W):
    x = np.asarray(x, np.float32)
    W = np.asarray(W, np.float32)
    in_maps = core_in_maps(x, W)
    nc = _build_nc()
    res = run_bass_kernel_spmd(nc, in_maps, list(range(N_CORES)))
    out = np.concatenate([res.results[c]["v"] for c in range(N_CORES)], axis=0)
    return out.astype(np.float32)


# revision 63
# speedup vs baseline: 1.9038x; 1.2822x over previous
"""DigitCaps dynamic-routing kernel for Trainium2 (8 NeuronCores, Bass/Tile).

Problem: B=256, IN_CAPS=3200, IN_DIM=8, OUT_CAPS=8, OUT_DIM=16, 3 routing
iterations.  Data-parallel over batch: 32 batches per core.

v2 design (per core):
  - batch processed in 8 "eighths" of 4 batches; per eighth, u_hat is
    created ONCE in SBUF bf16 in BOTH layouts and reused by both routing
    iterations (the baseline recreated it per iteration):
      u_jm [jm=128p, t, b, i]  via K=128 block-diagonal creation matmuls
      u_res [i=128p, t, b, jm] via XBAR DMA-transpose of u_jm tiles
  - creation runs per PAIR of eighths (8 batches): one 128-col-moving
    matmul per K-chunk halves the LDWEIGHTS-bound matmul count; the
    zero-padded x operand streams from DRAM on the sync ring only, with
    a 6-tile prefetch issued during the previous pair's routing.  XBAR
    transposes ride the scalar ring exclusively: mixing stream DMAs and
    XBARs across rings corrupts DMA-completion tracking on HW.
  - a-pass: stationary u_jm tile (128x128 bf16), moving block-diagonal
    v (8/16 cols) -> a^T [i-part, j] so softmax over j is a free-axis
    op on 128 partitions; softmax batched over 8-tile groups.
  - s-pass: stationary c columns (cheap 8-col weight loads), moving
    u_res tiles, accumulating s_ps [J, b, jm] over t with a mask/ones
    extraction (baseline orientation).
  - PSUM->SBUF eviction of created u_hat in 2-tile batches split 2:3
    over DVE / Activation.
  - emission is software-pipelined across eighths so the PE keeps
    working (next eighth's creation) while squash chains run on vector.
"""

import sys

if "/opt/trn_rl_repo" not in sys.path:
    sys.path.insert(0, "/opt/trn_rl_repo")

import ml_dtypes
import numpy as np

import bass_rust
import concourse.bass as bass
import concourse.mybir as mybir
import concourse.tile as tile
from concourse._compat import with_exitstack
from concourse.bass_utils import run_bass_kernel_spmd
from concourse.vector_clock import ScopedClock

# ---------------------------------------------------------------------------
# Walrus on this toolchain rejects multi-wait CTRL instructions;
# TileContext's tail drain aggregates one wait per outstanding semaphore.
# Split the waits across consecutive SP drains.
_TILE_PATCHED = False


def _drain_and_barrier_split(self, tick_clock, wait_clock):
    drain_inst = self.nc.sync.drain()
    wait_clock.add_sem_waits(
        drain_inst.ins, ScopedClock({None: tick_clock.global_clock})
    )
    mi = drain_inst.ins
    waits = list(mi.sync_info.on_wait) if mi.sync_info else []
    if len(waits) > 1:
        si = mi.sync_info
        si.on_wait = waits[:1]
        mi.sync_info = si
        for i in range(1, len(waits)):
            extra = self.nc.sync.drain().ins
            extra.sync_info = bass_rust.SyncInfo(
                on_wait=waits[i : i + 1], on_update=[]
            )
    self.nc.all_engine_barrier()
    assert self.sems is not None
    popped = self.nc._tile_sem_poison_stack.pop()
    assert popped is self._sem_poison
    self.nc.clear_and_free_semaphores(list(self.sems.allocated().values()))
    self.nc.all_engine_barrier()


def _patch_tile():
    global _TILE_PATCHED
    if not _TILE_PATCHED:
        tile.TileContext._drain_and_barrier = _drain_and_barrier_split
        _TILE_PATCHED = True


_SW_COUNT = [0]


def _split_waits(nc):
    """This walrus build allows one sync wait per instruction: hoist extra
    waits onto same-engine NoOp carriers placed just before."""
    for f in nc.m.functions:
        for blk in f.blocks:
            insts = blk.instructions
            if not any(
                inst.sync_info and len(inst.sync_info.on_wait) > 1
                for inst in insts
            ):
                continue
            new = []
            for inst in insts:
                si = inst.sync_info
                waits = list(si.on_wait) if si else []
                if len(waits) > 1:
                    for w in waits[:-1]:
                        _SW_COUNT[0] += 1
                        car = mybir.InstNoOp(
                            name=f"I-sw{_SW_COUNT[0]}", engine=inst.engine
                        )
                        car.sync_info = bass_rust.SyncInfo(
                            on_wait=[w], on_update=[]
                        )
                        new.append(car)
                    si.on_wait = waits[-1:]
                    inst.sync_info = si
                new.append(inst)
            insts[:] = new


# ---------------------------------------------------------------------------
B, I, N, J, M = 256, 3200, 8, 8, 16
JM = J * M  # 128
N_CORES = 8
B_C = B // N_CORES  # 32
T = I // 128  # 25 i-tiles

IP = 16  # i's packed per K-chunk (K = IP*N = 128, uniform row group)
H = I // IP  # 200
CH_T = 128 // IP  # 8 creation chunks per 128-i tile

E = 4  # batches per eighth
NE = B_C // E  # 8 eighths

F32 = mybir.dt.float32
BF16 = mybir.dt.bfloat16


def _squash_chain(nc, small, ps, s_sb, ident, nb):
    """s_sb [128(jm), nb] f32 -> vT [nb, 128] f32.
    squash per capsule j: sq = sum_m s^2, v = sq*s/((1+sq)*sqrt(sq))."""
    sT_ps = ps.tile([nb, JM], F32, tag="sx")
    nc.tensor.matmul(sT_ps[:], s_sb[:], ident[:], is_transpose=True)
    sT = small.tile([nb, J, M], F32, tag="sT")
    nc.vector.tensor_copy(sT[:], sT_ps[:].rearrange("b (j m) -> b j m", m=M))
    s2 = small.tile([nb, J, M], F32, tag="s2")
    nc.vector.tensor_tensor(s2[:], sT[:], sT[:], mybir.AluOpType.mult)
    sq = small.tile([nb, J], F32, tag="sq")
    nc.vector.tensor_reduce(sq[:], s2[:], mybir.AxisListType.X, mybir.AluOpType.add)
    rt = small.tile([nb, J], F32, tag="rt")
    nc.scalar.activation(rt[:], sq[:], mybir.ActivationFunctionType.Sqrt)
    den = small.tile([nb, J], F32, tag="den")
    nc.vector.tensor_scalar_add(den[:], sq[:], 1.0)
    nc.vector.tensor_tensor(den[:], den[:], rt[:], mybir.AluOpType.mult)
    rden = small.tile([nb, J], F32, tag="rden")
    nc.vector.reciprocal(rden[:], den[:])
    scale = small.tile([nb, J], F32, tag="scale")
    nc.vector.tensor_tensor(scale[:], sq[:], rden[:], mybir.AluOpType.mult)
    vT = small.tile([nb, J, M], F32, tag="vT")
    scale_b = bass.AP(
        scale.tensor, scale[:].offset, [scale[:].ap[0], scale[:].ap[1], [0, M]]
    )
    nc.vector.tensor_tensor(vT[:], sT[:], scale_b, mybir.AluOpType.mult)
    return vT


def _vblk_from_vT(nc, small, vT, mask_rep, nb):
    """vT [nb, 128] f32 -> vblk [128(jm), nb, J] bf16 block-diagonal over j.
    Uses the XBAR transpose; only safe for nb >= 32."""
    vT16 = small.tile([nb, JM], BF16, tag="vT16")
    nc.vector.tensor_copy(vT16[:], vT[:])
    vjm = small.tile([JM, nb], BF16, tag="vjm")
    nc.sync.dma_start_transpose(vjm[:], vT16[:])
    vblk = small.tile([JM, nb, J], BF16, tag="vblk_tmp")
    vjm_b = bass.AP(vjm.tensor, vjm[:].offset, [vjm[:].ap[0], vjm[:].ap[1], [0, J]])
    mask_b = bass.AP(
        mask_rep.tensor,
        mask_rep[:].offset,
        [mask_rep[:].ap[0], [0, nb], mask_rep[:].ap[1]],
    )
    nc.vector.tensor_tensor(vblk[:], vjm_b, mask_b, mybir.AluOpType.mult)
    return vblk


@with_exitstack
def build_kernel(ctx, tc, outs, ins, reps=1, stage=3):
    nc = tc.nc
    (v_out,) = outs
    (wcr_d, xblk_d, xt_d, mask_d, ident_d) = ins

    TG = 8  # t-group size for batched softmax

    const = ctx.enter_context(tc.tile_pool(name="const", bufs=1))
    ujmp = ctx.enter_context(tc.tile_pool(name="ujmp", bufs=2))
    uresp = ctx.enter_context(tc.tile_pool(name="uresp", bufs=2))
    sm = ctx.enter_context(tc.tile_pool(name="sm", bufs=2))
    ctp = ctx.enter_context(tc.tile_pool(name="ctp", bufs=8))
    small = ctx.enter_context(tc.tile_pool(name="small", bufs=2))
    psq = ctx.enter_context(tc.tile_pool(name="psq", bufs=1, space="PSUM"))
    cpsp = ctx.enter_context(tc.tile_pool(name="cpsp", bufs=2, space="PSUM"))
    apsp = ctx.enter_context(tc.tile_pool(name="apsp", bufs=2, space="PSUM"))
    saccp = ctx.enter_context(tc.tile_pool(name="saccp", bufs=1, space="PSUM"))

    # Resident constants.  wcr/xt split into chunked DMAs so early matmuls
    # can start before the whole tensor lands.
    wcr = const.tile([128, H, JM], BF16)
    for c4 in range(4):
        nc.sync.dma_start(wcr[:, c4 * 50 : (c4 + 1) * 50, :],
                          wcr_d[:, c4 * 50 : (c4 + 1) * 50, :])
    xt = const.tile([128, H, B_C], BF16)
    nc.scalar.dma_start(xt[:], xt_d[:])
    mask_rep = const.tile([JM, J], BF16)
    nc.sync.dma_start(mask_rep[:], mask_d[:])
    maskT = const.tile([J, JM], BF16)
    nc.sync.dma_start(maskT[:], mask_d[:].rearrange("a b -> b a"))
    ones8 = const.tile([J, 1], BF16)
    nc.vector.memset(ones8[:], 1.0)
    ident = const.tile([128, 128], F32)
    nc.sync.dma_start(ident[:], ident_d[:])

    xspa = ctx.enter_context(tc.tile_pool(name="xspa", bufs=3))
    xspb = ctx.enter_context(tc.tile_pool(name="xspb", bufs=3))
    TSPLIT = 13  # t < TSPLIT streams on the sync ring, rest on scalar
    PF = 3  # per-ring prefetch depth, DMA'd during the previous routing

    def prefetch(bat):
        tiles = {}
        for t in range(PF):
            xb = xspa.tile([128, CH_T, 2 * E, IP], BF16, tag="xba")
            nc.sync.dma_start(xb[:], xblk_d[bat, t])
            tiles[t] = xb
        for t in range(TSPLIT, TSPLIT + PF):
            xb = xspb.tile([128, CH_T, 2 * E, IP], BF16, tag="xbb")
            nc.scalar.dma_start(xb[:], xblk_d[bat, t])
            tiles[t] = xb
        return tiles

    def create2(bat, pf, ujm0, ures0, ujm1, ures1, s1_ps=None):
        """Creation for one 8-batch chunk (eighths 2*bat, 2*bat+1): one
        128-col-moving matmul per h halves the LDWEIGHTS-bound matmul
        count; evictions split per eighth across DVE/Act.  All xb stream
        DMAs ride the sync ring and all XBAR transposes the scalar ring:
        mixing them across rings corrupts DMA-completion tracking on HW
        (out-of-order completions on shared semaphore lanes)."""
        for t in range(T):
            if t in pf:
                xb = pf[t]
            elif t < TSPLIT:
                xb = xspa.tile([128, CH_T, 2 * E, IP], BF16, tag="xba")
                nc.sync.dma_start(xb[:], xblk_d[bat, t])
            else:
                xb = xspb.tile([128, CH_T, 2 * E, IP], BF16, tag="xbb")
                nc.scalar.dma_start(xb[:], xblk_d[bat, t])
            cps = cpsp.tile([JM, CH_T, 2 * E, IP], F32, tag="cps")
            for g in range(CH_T):
                h = t * CH_T + g
                nc.tensor.matmul(
                    cps[:, g, :, :], wcr[:, h, :], xb[:, g, :, :],
                    start=True, stop=True,
                )
                if s1_ps is not None:
                    # iteration-1 partial with the SAME stationary wcr[:, h]
                    # as the adjacent creation matmul (weight-load reuse)
                    nc.tensor.matmul(
                        s1_ps[:], wcr[:, h, :], xt[:, h, :],
                        start=False, stop=False, skip_group_check=True,
                    )
            for half, ujm in ((0, ujm0), (1, ujm1)):
                dst = ujm[:, t, :, :].rearrange("p b (g i) -> p g b i", i=IP)
                src = cps[:, :, half * E : (half + 1) * E, :]
                if (t + half) % 2 == 0:
                    nc.vector.tensor_copy(dst, src)
                else:
                    nc.scalar.activation(
                        dst, src, mybir.ActivationFunctionType.Copy
                    )
            if t % 5 == 4:
                for half, (ujm, ures) in enumerate(
                    ((ujm0, ures0), (ujm1, ures1))
                ):
                    nc.scalar.dma_start_transpose(
                        ures[:, t - 4 : t + 1, :, :],
                        ujm[:, t - 4 : t + 1, :, :],
                    )

    def apass_softmax(q, it, ujm, vblk):
        """a-pass + batched softmax for eighth q, iteration it (2|3).
        Returns list of c_t tiles [128(i), TG, E, J] bf16 per t-group."""
        nslot = it - 1
        cts = []
        for g0 in range(0, T, TG):
            g1 = min(g0 + TG, T)
            ng = g1 - g0
            aps = apsp.tile([128, TG, E, 16], F32, tag="aps")
            for t in range(g0, g1):
                for b in range(E):
                    nc.tensor.matmul(
                        aps[:, t - g0, b, : nslot * J],
                        ujm[:, t, b, :],
                        vblk[:, q * E + b, :nslot, :],
                        start=True, stop=True,
                    )
            av = aps[:, :ng]
            e = sm.tile([128, TG, E, J], BF16, tag="e")
            if it == 2:
                # exp straight from PSUM; no logits copy needed
                nc.scalar.activation(
                    e[:, :ng], av[:, :, :, 0:J],
                    mybir.ActivationFunctionType.Exp,
                )
            else:
                lg0 = sm.tile([128, TG, E, J], F32, tag="lg0")
                nc.scalar.activation(
                    lg0[:, :ng], av[:, :, :, 0:J],
                    mybir.ActivationFunctionType.Copy,
                )
                lg = sm.tile([128, TG, E, J], F32, tag="lg")
                nc.vector.tensor_tensor(
                    lg[:, :ng], lg0[:, :ng], av[:, :, :, J : 2 * J],
                    mybir.AluOpType.add,
                )
                nc.scalar.activation(
                    e[:, :ng], lg[:, :ng], mybir.ActivationFunctionType.Exp
                )
            z = sm.tile([128, TG, E], F32, tag="z")
            nc.vector.tensor_reduce(
                z[:, :ng], e[:, :ng], mybir.AxisListType.X, mybir.AluOpType.add
            )
            rz = sm.tile([128, TG, E], F32, tag="rz")
            nc.vector.reciprocal(rz[:, :ng], z[:, :ng])
            c_t = ctp.tile([128, TG, E, J], BF16, tag="c_t")
            rzb = bass.AP(
                rz.tensor, rz[:, :ng].offset,
                [rz[:].ap[0], [rz[:].ap[1][0], ng], rz[:].ap[2], [0, J]],
            )
            nc.vector.tensor_tensor(
                c_t[:, :ng], e[:, :ng], rzb, mybir.AluOpType.mult
            )
            cts.append(c_t)
        return cts

    def spass(q, it, ures, cts):
        """s-pass: stationary c columns (cheap weight loads), moving u_res
        tiles; accumulates s_ps [J, E, JM] over t.  Returns s_sb [jm, E]."""
        sacc = saccp.tile([J, E, JM], F32, tag="sacc")
        nc.vector.memset(sacc[:], 0.0)
        for g0 in range(0, T, TG):
            c_t = cts[g0 // TG]
            for t in range(g0, min(g0 + TG, T)):
                for b in range(E):
                    nc.tensor.matmul(
                        sacc[:, b, :],
                        c_t[:, t - g0, b, :],
                        ures[:, t, b, :],
                        start=False, stop=False, skip_group_check=True,
                    )
        msb = small.tile([J, E, JM], BF16, tag="msb")
        maskT_b = bass.AP(
            maskT.tensor, maskT[:].offset,
            [maskT[:].ap[0], [0, E], maskT[:].ap[1]],
        )
        nc.vector.tensor_tensor(msb[:], sacc[:], maskT_b, mybir.AluOpType.mult)
        s2_ps = psq.tile([JM, E], F32, tag="sx")
        for b in range(E):
            nc.tensor.matmul(
                s2_ps[:, b : b + 1], msb[:, b, :], ones8[:],
                start=True, stop=True,
            )
        s_sb = small.tile([JM, E], F32, tag="s_sb")
        nc.vector.tensor_copy(s_sb[:], s2_ps[:])
        return s_sb

    def vblk_write(q, vTh, vblk):
        """vTh [E, 128] f32 -> vblk[:, q*E:(q+1)*E, 1, :] via PE transpose."""
        vps = psq.tile([JM, E], F32, tag="sx")
        nc.tensor.matmul(
            vps[:], vTh[:], ident[0:E, 0:E], is_transpose=True
        )
        vjm = small.tile([JM, E], BF16, tag="vjms")
        nc.vector.tensor_copy(vjm[:], vps[:])
        vjm_b = bass.AP(
            vjm.tensor, vjm[:].offset, [vjm[:].ap[0], vjm[:].ap[1], [0, J]]
        )
        mask_b = bass.AP(
            mask_rep.tensor, mask_rep[:].offset,
            [mask_rep[:].ap[0], [0, E], mask_rep[:].ap[1]],
        )
        nc.vector.tensor_tensor(
            vblk[:, q * E : (q + 1) * E, 1, :], vjm_b, mask_b,
            mybir.AluOpType.mult,
        )

    vblk = const.tile([JM, B_C, 2, J], BF16, tag="vblk")

    for rep in range(reps):
        # iteration-1 accumulator; filled during pair-0 creation (shared
        # wcr stationaries)
        s1_ps = psq.tile([JM, B_C], F32, tag="sx")
        nc.vector.memset(s1_ps[:], 0.0)

        ujm = [None] * NE
        ures = [None] * NE

        def mk_pair(bat, pf, s1=None):
            for q in (2 * bat, 2 * bat + 1):
                uj = ujmp.tile([JM, T, E, 128], BF16, tag="ujm")
                ur = uresp.tile([128, T, E, JM], BF16, tag="ures")
                ujm[q] = uj
                ures[q] = ur
            create2(bat, pf, ujm[2 * bat], ures[2 * bat],
                    ujm[2 * bat + 1], ures[2 * bat + 1], s1_ps=s1)

        mk_pair(0, prefetch(0), s1_ps)
        # ---- iteration 1 (all batches): s1 = (1/8) sum_(i,n) W x ----------
        s_sb = small.tile([JM, B_C], F32, tag="s_all")
        nc.vector.tensor_scalar_mul(s_sb[:], s1_ps[:], 1.0 / J)
        vT = _squash_chain(nc, small, psq, s_sb, ident, B_C)
        vb = _vblk_from_vT(nc, small, vT, mask_rep, B_C)
        nc.vector.tensor_copy(vblk[:, :, 0, :], vb[:])
        if stage < 3:
            # ablation: creation only
            for bat in range(1, NE // 2):
                mk_pair(bat, prefetch(bat))
            nc.sync.dma_start(
                v_out[:].rearrange("b j m -> b (j m)")[:, :], vT[:]
            )
            continue

        cts2 = apass_softmax(0, 2, ujm[0], vblk)
        pf = None
        for q in range(NE):
            if q % 2 == 0 and q + 2 < NE:
                # stream the next pair's first xb tiles during this routing
                pf = prefetch(q // 2 + 1)
            # it2 finish: s-pass, squash, vblk slot 1
            s_sb2 = spass(q, 2, ures[q], cts2)
            vT2 = _squash_chain(nc, small, psq, s_sb2, ident, E)
            vblk_write(q, vT2, vblk)
            cts3 = apass_softmax(q, 3, ujm[q], vblk)
            if q % 2 == 0:
                cts2 = apass_softmax(q + 1, 2, ujm[q + 1], vblk)
            s_sb3 = spass(q, 3, ures[q], cts3)
            vT3 = _squash_chain(nc, small, psq, s_sb3, ident, E)
            nc.sync.dma_start(
                v_out[:].rearrange("b j m -> b (j m)")[q * E : (q + 1) * E, :],
                vT3[:],
            )
            if q % 2 == 1 and q + 1 < NE:
                # next pair's creation; all readers of this pair's u done
                mk_pair((q + 1) // 2, pf)
                cts2 = apass_softmax(q + 1, 2, ujm[q + 1], vblk)


_NC_CACHE = {}


def _build_nc(reps=1, stage=3):
    key = (reps, stage)
    if key not in _NC_CACHE:
        _patch_tile()
        nc = bass.Bass("TRN2", target_bir_lowering=False, debug=False)
        wcr_d = nc.dram_tensor("wcr", [128, H, JM], BF16, kind="ExternalInput").ap()
        xblk_d = nc.dram_tensor(
            "xblk", [NE // 2, T, 128, CH_T, 2 * E, IP], BF16,
            kind="ExternalInput",
        ).ap()
        xt_d = nc.dram_tensor("xt", [128, H, B_C], BF16, kind="ExternalInput").ap()
        mask_d = nc.dram_tensor("mask", [JM, J], BF16, kind="ExternalInput").ap()
        ident_d = nc.dram_tensor("ident", [128, 128], F32, kind="ExternalInput").ap()
        v_d = nc.dram_tensor("v", [B_C, J, M], F32, kind="ExternalOutput").ap()
        with tile.TileContext(nc) as tc:
            build_kernel(
                tc,
                [v_d],
                [wcr_d, xblk_d, xt_d, mask_d, ident_d],
                reps=reps,
                stage=stage,
            )
        _split_waits(nc)
        _NC_CACHE[key] = nc
    return _NC_CACHE[key]


def host_prep(x, W):
    """Returns (wcr, xdg_all, xt_all, mask, ident); x-deriveds cover all B.
    Row order of the 128 K-rows is (i16, n): i = h*IP + i16."""
    bf = ml_dtypes.bfloat16
    nb = x.shape[0]
    # wcr[(i16*N + n), h, jm] = W[h*IP + i16, j, n, m]
    Wr = np.ascontiguousarray(W.transpose(0, 2, 1, 3)).reshape(I, N, JM)
    Wr = Wr.reshape(H, IP, N, JM)
    wcr = np.ascontiguousarray(Wr.transpose(1, 2, 0, 3)).reshape(128, H, JM)
    # x rows in the same (i16, n) order per h
    xr = x.reshape(nb, H, IP, N)
    xrows = np.ascontiguousarray(xr.transpose(2, 3, 1, 0)).reshape(128, H, nb)
    # zero-padded block-diagonal x operand rows
    rows = np.arange(128)
    i16_of_row = rows // N
    xblk = np.zeros((128, H, nb, IP), np.float32)
    for r in range(128):
        xblk[r, :, :, i16_of_row[r]] = xrows[r]
    mask = np.zeros((JM, J), np.float32)
    for j in range(J):
        mask[j * M : (j + 1) * M, j] = 1.0
    ident = np.eye(128, dtype=np.float32)
    return (
        wcr.astype(bf),
        xblk.astype(bf),
        xrows.astype(bf),
        mask.astype(bf),
        ident,
    )


def regroup(xblk_core):
    """xblk [128,H,nb,IP] -> [nb//8, T, 128, CH_T, 8, IP]."""
    nb = xblk_core.shape[2]
    xb = xblk_core.reshape(128, T, CH_T, nb, IP)
    xb = xb.transpose(3, 1, 0, 2, 4)  # [nb, T, 128, CH_T, IP]
    xb = xb.reshape(nb // 8, 8, T, 128, CH_T, IP).transpose(0, 2, 3, 4, 1, 5)
    return np.ascontiguousarray(xb)


def core_in_maps(x, W):
    """Per-core input dicts for run_bass_kernel_spmd."""
    wcr, xblk_all, xt_all, mask, ident = host_prep(x, W)
    in_maps = []
    for c in range(N_CORES):
        bs = slice(c * B_C, (c + 1) * B_C)
        in_maps.append(
            {
                "wcr": wcr,
                "xblk": regroup(xblk_all[:, :, bs, :]),
                "xt": np.ascontiguousarray(xt_all[:, :, bs]),
                "mask": mask,
                "ident": ident,
            }
        )
    return in_maps


def kernel(x, W):
    x = np.asarray(x, np.float32)
    W = np.asarray(W, np.float32)
    in_maps = core_in_maps(x, W)
    nc = _build_nc()
    res = run_bass_kernel_spmd(nc, in_maps, list(range(N_CORES)))
    out = np.concatenate([res.results[c]["v"] for c in range(N_CORES)], axis=0)
    return out.astype(np.float32)


# revision 64
# speedup vs baseline: 3.4803x; 1.8281x over previous
"""DigitCaps dynamic-routing kernel for Trainium2 (8 NeuronCores, Bass/Tile).

Problem: B=256, IN_CAPS=3200, IN_DIM=8, OUT_CAPS=8, OUT_DIM=16, 3 routing
iterations.  Data-parallel over batch: 32 batches per core.

v2 design (per core):
  - batch processed in 8 "eighths" of 4 batches; per eighth, u_hat is
    created ONCE in SBUF bf16 in BOTH layouts and reused by both routing
    iterations (the baseline recreated it per iteration):
      u_jm [jm=128p, t, b, i]  via K=128 block-diagonal creation matmuls
      u_res [i=128p, t, b, jm] via XBAR DMA-transpose of u_jm tiles
  - creation runs per PAIR of eighths (8 batches): one 128-col-moving
    matmul per K-chunk halves the LDWEIGHTS-bound matmul count; the
    zero-padded x operand streams from DRAM on the sync ring only, with
    a 6-tile prefetch issued during the previous pair's routing.  XBAR
    transposes ride the scalar ring exclusively: mixing stream DMAs and
    XBARs across rings corrupts DMA-completion tracking on HW.
  - a-pass: stationary u_jm tile (128x128 bf16), moving block-diagonal
    v (8/16 cols) -> a^T [i-part, j] so softmax over j is a free-axis
    op on 128 partitions; softmax batched over 8-tile groups.
  - s-pass: stationary c columns (cheap 8-col weight loads), moving
    u_res tiles, accumulating s_ps [J, b, jm] over t with a mask/ones
    extraction (baseline orientation).
  - PSUM->SBUF eviction of created u_hat in 2-tile batches split 2:3
    over DVE / Activation.
  - emission is software-pipelined across eighths so the PE keeps
    working (next eighth's creation) while squash chains run on vector.
"""

import sys

if "/opt/trn_rl_repo" not in sys.path:
    sys.path.insert(0, "/opt/trn_rl_repo")

import ml_dtypes
import numpy as np

import bass_rust
import concourse.bass as bass
import concourse.mybir as mybir
import concourse.tile as tile
from concourse._compat import with_exitstack
from concourse.bass_utils import run_bass_kernel_spmd
from concourse.vector_clock import ScopedClock

# ---------------------------------------------------------------------------
# Walrus on this toolchain rejects multi-wait CTRL instructions;
# TileContext's tail drain aggregates one wait per outstanding semaphore.
# Split the waits across consecutive SP drains.
_TILE_PATCHED = False


def _drain_and_barrier_split(self, tick_clock, wait_clock):
    drain_inst = self.nc.sync.drain()
    wait_clock.add_sem_waits(
        drain_inst.ins, ScopedClock({None: tick_clock.global_clock})
    )
    mi = drain_inst.ins
    waits = list(mi.sync_info.on_wait) if mi.sync_info else []
    if len(waits) > 1:
        si = mi.sync_info
        si.on_wait = waits[:1]
        mi.sync_info = si
        for i in range(1, len(waits)):
            extra = self.nc.sync.drain().ins
            extra.sync_info = bass_rust.SyncInfo(
                on_wait=waits[i : i + 1], on_update=[]
            )
    self.nc.all_engine_barrier()
    assert self.sems is not None
    popped = self.nc._tile_sem_poison_stack.pop()
    assert popped is self._sem_poison
    self.nc.clear_and_free_semaphores(list(self.sems.allocated().values()))
    self.nc.all_engine_barrier()


def _patch_tile():
    global _TILE_PATCHED
    if not _TILE_PATCHED:
        tile.TileContext._drain_and_barrier = _drain_and_barrier_split
        _TILE_PATCHED = True


_SW_COUNT = [0]


def _split_waits(nc):
    """This walrus build allows one sync wait per instruction: hoist extra
    waits onto same-engine NoOp carriers placed just before."""
    for f in nc.m.functions:
        for blk in f.blocks:
            insts = blk.instructions
            if not any(
                inst.sync_info and len(inst.sync_info.on_wait) > 1
                for inst in insts
            ):
                continue
            new = []
            for inst in insts:
                si = inst.sync_info
                waits = list(si.on_wait) if si else []
                if len(waits) > 1:
                    for w in waits[:-1]:
                        _SW_COUNT[0] += 1
                        car = mybir.InstNoOp(
                            name=f"I-sw{_SW_COUNT[0]}", engine=inst.engine
                        )
                        car.sync_info = bass_rust.SyncInfo(
                            on_wait=[w], on_update=[]
                        )
                        new.append(car)
                    si.on_wait = waits[-1:]
                    inst.sync_info = si
                new.append(inst)
            insts[:] = new


# ---------------------------------------------------------------------------
B, I, N, J, M = 256, 3200, 8, 8, 16
JM = J * M  # 128
N_CORES = 8
B_C = B // N_CORES  # 32
T = I // 128  # 25 i-tiles

IP = 16  # i's packed per K-chunk (K = IP*N = 128, uniform row group)
H = I // IP  # 200
CH_T = 128 // IP  # 8 creation chunks per 128-i tile

E = 4  # batches per eighth
NE = B_C // E  # 8 eighths

F32 = mybir.dt.float32
BF16 = mybir.dt.bfloat16


def _squash_chain(nc, small, ps, s_sb, ident, nb):
    """s_sb [128(jm), nb] f32 -> vT [nb, 128] f32.
    squash per capsule j: sq = sum_m s^2, v = sq*s/((1+sq)*sqrt(sq))."""
    sT_ps = ps.tile([nb, JM], F32, tag="sx")
    nc.tensor.matmul(sT_ps[:], s_sb[:], ident[:], is_transpose=True)
    sT = small.tile([nb, J, M], F32, tag="sT")
    nc.vector.tensor_copy(sT[:], sT_ps[:].rearrange("b (j m) -> b j m", m=M))
    s2 = small.tile([nb, J, M], F32, tag="s2")
    nc.vector.tensor_tensor(s2[:], sT[:], sT[:], mybir.AluOpType.mult)
    sq = small.tile([nb, J], F32, tag="sq")
    nc.vector.tensor_reduce(sq[:], s2[:], mybir.AxisListType.X, mybir.AluOpType.add)
    rt = small.tile([nb, J], F32, tag="rt")
    nc.scalar.activation(rt[:], sq[:], mybir.ActivationFunctionType.Sqrt)
    den = small.tile([nb, J], F32, tag="den")
    nc.vector.tensor_scalar_add(den[:], sq[:], 1.0)
    nc.vector.tensor_tensor(den[:], den[:], rt[:], mybir.AluOpType.mult)
    rden = small.tile([nb, J], F32, tag="rden")
    nc.vector.reciprocal(rden[:], den[:])
    scale = small.tile([nb, J], F32, tag="scale")
    nc.vector.tensor_tensor(scale[:], sq[:], rden[:], mybir.AluOpType.mult)
    vT = small.tile([nb, J, M], F32, tag="vT")
    scale_b = bass.AP(
        scale.tensor, scale[:].offset, [scale[:].ap[0], scale[:].ap[1], [0, M]]
    )
    nc.vector.tensor_tensor(vT[:], sT[:], scale_b, mybir.AluOpType.mult)
    return vT


def _vblk_from_vT(nc, small, vT, mask_rep, nb):
    """vT [nb, 128] f32 -> vblk [128(jm), nb, J] bf16 block-diagonal over j.
    Uses the XBAR transpose; only safe for nb >= 32."""
    vT16 = small.tile([nb, JM], BF16, tag="vT16")
    nc.vector.tensor_copy(vT16[:], vT[:])
    vjm = small.tile([JM, nb], BF16, tag="vjm")
    nc.sync.dma_start_transpose(vjm[:], vT16[:])
    vblk = small.tile([JM, nb, J], BF16, tag="vblk_tmp")
    vjm_b = bass.AP(vjm.tensor, vjm[:].offset, [vjm[:].ap[0], vjm[:].ap[1], [0, J]])
    mask_b = bass.AP(
        mask_rep.tensor,
        mask_rep[:].offset,
        [mask_rep[:].ap[0], [0, nb], mask_rep[:].ap[1]],
    )
    nc.vector.tensor_tensor(vblk[:], vjm_b, mask_b, mybir.AluOpType.mult)
    return vblk


@with_exitstack
def build_kernel(ctx, tc, outs, ins, reps=1, stage=3):
    nc = tc.nc
    (v_out,) = outs
    (wcr_d, xblk_d, xt_d, mask_d, ident_d) = ins

    TG = 8  # t-group size for batched softmax

    const = ctx.enter_context(tc.tile_pool(name="const", bufs=1))
    ujmp = ctx.enter_context(tc.tile_pool(name="ujmp", bufs=2))
    uresp = ctx.enter_context(tc.tile_pool(name="uresp", bufs=2))
    sm = ctx.enter_context(tc.tile_pool(name="sm", bufs=2))
    ctp = ctx.enter_context(tc.tile_pool(name="ctp", bufs=8))
    small = ctx.enter_context(tc.tile_pool(name="small", bufs=2))
    psq = ctx.enter_context(tc.tile_pool(name="psq", bufs=1, space="PSUM"))
    cpsp = ctx.enter_context(tc.tile_pool(name="cpsp", bufs=2, space="PSUM"))
    apsp = ctx.enter_context(tc.tile_pool(name="apsp", bufs=2, space="PSUM"))
    saccp = ctx.enter_context(tc.tile_pool(name="saccp", bufs=1, space="PSUM"))

    # Resident constants.  wcr/xt split into chunked DMAs so early matmuls
    # can start before the whole tensor lands.
    wcr = const.tile([128, H, JM], BF16)
    for c4 in range(4):
        nc.sync.dma_start(wcr[:, c4 * 50 : (c4 + 1) * 50, :],
                          wcr_d[:, c4 * 50 : (c4 + 1) * 50, :])
    xt = const.tile([128, H, B_C], BF16)
    nc.scalar.dma_start(xt[:], xt_d[:])
    mask_rep = const.tile([JM, J], BF16)
    nc.sync.dma_start(mask_rep[:], mask_d[:])
    maskT = const.tile([J, JM], BF16)
    nc.sync.dma_start(maskT[:], mask_d[:].rearrange("a b -> b a"))
    ones8 = const.tile([J, 1], BF16)
    nc.vector.memset(ones8[:], 1.0)
    ident = const.tile([128, 128], F32)
    nc.sync.dma_start(ident[:], ident_d[:])

    xspa = ctx.enter_context(tc.tile_pool(name="xspa", bufs=3))
    xspb = ctx.enter_context(tc.tile_pool(name="xspb", bufs=3))
    # t < TSPLIT streams on the sync ring, rest on scalar; 16/9 balances
    # the rings since scalar also carries the 10 XBAR transposes per pair
    TSPLIT = 16
    PF = 3  # per-ring prefetch depth, DMA'd during the previous routing

    def prefetch(bat):
        tiles = {}
        for t in range(PF):
            xb = xspa.tile([128, CH_T, 2 * E, IP], BF16, tag="xba")
            nc.sync.dma_start(xb[:], xblk_d[bat, t])
            tiles[t] = xb
        for t in range(TSPLIT, TSPLIT + PF):
            xb = xspb.tile([128, CH_T, 2 * E, IP], BF16, tag="xbb")
            nc.scalar.dma_start(xb[:], xblk_d[bat, t])
            tiles[t] = xb
        return tiles

    def create2(bat, pf, ujm0, ures0, ujm1, ures1, s1_ps=None):
        """Creation for one 8-batch chunk (eighths 2*bat, 2*bat+1): one
        128-col-moving matmul per h halves the LDWEIGHTS-bound matmul
        count; evictions split per eighth across DVE/Act.  All xb stream
        DMAs ride the sync ring and all XBAR transposes the scalar ring:
        mixing them across rings corrupts DMA-completion tracking on HW
        (out-of-order completions on shared semaphore lanes)."""
        for t in range(T):
            if t in pf:
                xb = pf[t]
            elif t < TSPLIT:
                xb = xspa.tile([128, CH_T, 2 * E, IP], BF16, tag="xba")
                nc.sync.dma_start(xb[:], xblk_d[bat, t])
            else:
                xb = xspb.tile([128, CH_T, 2 * E, IP], BF16, tag="xbb")
                nc.scalar.dma_start(xb[:], xblk_d[bat, t])
            cps = cpsp.tile([JM, CH_T, 2 * E, IP], F32, tag="cps")
            for g in range(CH_T):
                h = t * CH_T + g
                nc.tensor.matmul(
                    cps[:, g, :, :], wcr[:, h, :], xb[:, g, :, :],
                    start=True, stop=True,
                )
                if s1_ps is not None:
                    # iteration-1 partial with the SAME stationary wcr[:, h]
                    # as the adjacent creation matmul (weight-load reuse)
                    nc.tensor.matmul(
                        s1_ps[:], wcr[:, h, :], xt[:, h, :],
                        start=False, stop=False, skip_group_check=True,
                    )
            for half, ujm in ((0, ujm0), (1, ujm1)):
                dst = ujm[:, t, :, :].rearrange("p b (g i) -> p g b i", i=IP)
                src = cps[:, :, half * E : (half + 1) * E, :]
                if (t + half) % 2 == 0:
                    nc.vector.tensor_copy(dst, src)
                else:
                    nc.scalar.activation(
                        dst, src, mybir.ActivationFunctionType.Copy
                    )
            if t % 5 == 4:
                for half, (ujm, ures) in enumerate(
                    ((ujm0, ures0), (ujm1, ures1))
                ):
                    nc.scalar.dma_start_transpose(
                        ures[:, t - 4 : t + 1, :, :],
                        ujm[:, t - 4 : t + 1, :, :],
                    )

    def apass_softmax(q, it, ujm, vblk):
        """a-pass + batched softmax for eighth q, iteration it (2|3).
        Returns list of c_t tiles [128(i), TG, E, J] bf16 per t-group."""
        nslot = it - 1
        cts = []
        for g0 in range(0, T, TG):
            g1 = min(g0 + TG, T)
            ng = g1 - g0
            aps = apsp.tile([128, TG, E, 16], F32, tag="aps")
            for t in range(g0, g1):
                for b in range(E):
                    nc.tensor.matmul(
                        aps[:, t - g0, b, : nslot * J],
                        ujm[:, t, b, :],
                        vblk[:, q * E + b, :nslot, :],
                        start=True, stop=True,
                    )
            av = aps[:, :ng]
            e = sm.tile([128, TG, E, J], BF16, tag="e")
            if it == 2:
                # exp straight from PSUM; no logits copy needed
                nc.scalar.activation(
                    e[:, :ng], av[:, :, :, 0:J],
                    mybir.ActivationFunctionType.Exp,
                )
            else:
                lg0 = sm.tile([128, TG, E, J], F32, tag="lg0")
                nc.scalar.activation(
                    lg0[:, :ng], av[:, :, :, 0:J],
                    mybir.ActivationFunctionType.Copy,
                )
                lg = sm.tile([128, TG, E, J], F32, tag="lg")
                nc.vector.tensor_tensor(
                    lg[:, :ng], lg0[:, :ng], av[:, :, :, J : 2 * J],
                    mybir.AluOpType.add,
                )
                nc.scalar.activation(
                    e[:, :ng], lg[:, :ng], mybir.ActivationFunctionType.Exp
                )
            z = sm.tile([128, TG, E], F32, tag="z")
            nc.vector.tensor_reduce(
                z[:, :ng], e[:, :ng], mybir.AxisListType.X, mybir.AluOpType.add
            )
            rz = sm.tile([128, TG, E], F32, tag="rz")
            nc.vector.reciprocal(rz[:, :ng], z[:, :ng])
            c_t = ctp.tile([128, TG, E, J], BF16, tag="c_t")
            rzb = bass.AP(
                rz.tensor, rz[:, :ng].offset,
                [rz[:].ap[0], [rz[:].ap[1][0], ng], rz[:].ap[2], [0, J]],
            )
            nc.vector.tensor_tensor(
                c_t[:, :ng], e[:, :ng], rzb, mybir.AluOpType.mult
            )
            cts.append(c_t)
        return cts

    def spass(q, it, ures, cts):
        """s-pass: stationary c columns (cheap weight loads), moving u_res
        tiles; accumulates s_ps [J, E, JM] over t.  Returns s_sb [jm, E]."""
        sacc = saccp.tile([J, E, JM], F32, tag="sacc")
        nc.vector.memset(sacc[:], 0.0)
        for g0 in range(0, T, TG):
            c_t = cts[g0 // TG]
            for t in range(g0, min(g0 + TG, T)):
                for b in range(E):
                    nc.tensor.matmul(
                        sacc[:, b, :],
                        c_t[:, t - g0, b, :],
                        ures[:, t, b, :],
                        start=False, stop=False, skip_group_check=True,
                    )
        msb = small.tile([J, E, JM], BF16, tag="msb")
        maskT_b = bass.AP(
            maskT.tensor, maskT[:].offset,
            [maskT[:].ap[0], [0, E], maskT[:].ap[1]],
        )
        nc.vector.tensor_tensor(msb[:], sacc[:], maskT_b, mybir.AluOpType.mult)
        s2_ps = psq.tile([JM, E], F32, tag="sx")
        for b in range(E):
            nc.tensor.matmul(
                s2_ps[:, b : b + 1], msb[:, b, :], ones8[:],
                start=True, stop=True,
            )
        s_sb = small.tile([JM, E], F32, tag="s_sb")
        nc.vector.tensor_copy(s_sb[:], s2_ps[:])
        return s_sb

    def vblk_write(q, vTh, vblk):
        """vTh [E, 128] f32 -> vblk[:, q*E:(q+1)*E, 1, :] via PE transpose."""
        vps = psq.tile([JM, E], F32, tag="sx")
        nc.tensor.matmul(
            vps[:], vTh[:], ident[0:E, 0:E], is_transpose=True
        )
        vjm = small.tile([JM, E], BF16, tag="vjms")
        nc.vector.tensor_copy(vjm[:], vps[:])
        vjm_b = bass.AP(
            vjm.tensor, vjm[:].offset, [vjm[:].ap[0], vjm[:].ap[1], [0, J]]
        )
        mask_b = bass.AP(
            mask_rep.tensor, mask_rep[:].offset,
            [mask_rep[:].ap[0], [0, E], mask_rep[:].ap[1]],
        )
        nc.vector.tensor_tensor(
            vblk[:, q * E : (q + 1) * E, 1, :], vjm_b, mask_b,
            mybir.AluOpType.mult,
        )

    vblk = const.tile([JM, B_C, 2, J], BF16, tag="vblk")

    for rep in range(reps):
        # iteration-1 accumulator; filled during pair-0 creation (shared
        # wcr stationaries)
        s1_ps = psq.tile([JM, B_C], F32, tag="sx")
        nc.vector.memset(s1_ps[:], 0.0)

        ujm = [None] * NE
        ures = [None] * NE

        def mk_pair(bat, pf, s1=None):
            for q in (2 * bat, 2 * bat + 1):
                uj = ujmp.tile([JM, T, E, 128], BF16, tag="ujm")
                ur = uresp.tile([128, T, E, JM], BF16, tag="ures")
                ujm[q] = uj
                ures[q] = ur
            create2(bat, pf, ujm[2 * bat], ures[2 * bat],
                    ujm[2 * bat + 1], ures[2 * bat + 1], s1_ps=s1)

        mk_pair(0, prefetch(0), s1_ps)
        # ---- iteration 1 (all batches): s1 = (1/8) sum_(i,n) W x ----------
        s_sb = small.tile([JM, B_C], F32, tag="s_all")
        nc.vector.tensor_scalar_mul(s_sb[:], s1_ps[:], 1.0 / J)
        vT = _squash_chain(nc, small, psq, s_sb, ident, B_C)
        vb = _vblk_from_vT(nc, small, vT, mask_rep, B_C)
        nc.vector.tensor_copy(vblk[:, :, 0, :], vb[:])
        if stage < 3:
            # ablation: creation only
            for bat in range(1, NE // 2):
                mk_pair(bat, prefetch(bat))
            nc.sync.dma_start(
                v_out[:].rearrange("b j m -> b (j m)")[:, :], vT[:]
            )
            continue

        cts2 = apass_softmax(0, 2, ujm[0], vblk)
        pf = None
        for q in range(NE):
            if q % 2 == 0 and q + 2 < NE:
                # stream the next pair's first xb tiles during this routing
                pf = prefetch(q // 2 + 1)
            # it2 finish: s-pass, squash, vblk slot 1
            s_sb2 = spass(q, 2, ures[q], cts2)
            vT2 = _squash_chain(nc, small, psq, s_sb2, ident, E)
            vblk_write(q, vT2, vblk)
            cts3 = apass_softmax(q, 3, ujm[q], vblk)
            if q % 2 == 0:
                cts2 = apass_softmax(q + 1, 2, ujm[q + 1], vblk)
            s_sb3 = spass(q, 3, ures[q], cts3)
            vT3 = _squash_chain(nc, small, psq, s_sb3, ident, E)
            nc.sync.dma_start(
                v_out[:].rearrange("b j m -> b (j m)")[q * E : (q + 1) * E, :],
                vT3[:],
            )
            if q % 2 == 1 and q + 1 < NE:
                # next pair's creation; all readers of this pair's u done
                mk_pair((q + 1) // 2, pf)
                cts2 = apass_softmax(q + 1, 2, ujm[q + 1], vblk)


_NC_CACHE = {}


def _build_nc(reps=1, stage=3):
    key = (reps, stage)
    if key not in _NC_CACHE:
        _patch_tile()
        nc = bass.Bass("TRN2", target_bir_lowering=False, debug=False)
        wcr_d = nc.dram_tensor("wcr", [128, H, JM], BF16, kind="ExternalInput").ap()
        xblk_d = nc.dram_tensor(
            "xblk", [NE // 2, T, 128, CH_T, 2 * E, IP], BF16,
            kind="ExternalInput",
        ).ap()
        xt_d = nc.dram_tensor("xt", [128, H, B_C], BF16, kind="ExternalInput").ap()
        mask_d = nc.dram_tensor("mask", [JM, J], BF16, kind="ExternalInput").ap()
        ident_d = nc.dram_tensor("ident", [128, 128], F32, kind="ExternalInput").ap()
        v_d = nc.dram_tensor("v", [B_C, J, M], F32, kind="ExternalOutput").ap()
        with tile.TileContext(nc) as tc:
            build_kernel(
                tc,
                [v_d],
                [wcr_d, xblk_d, xt_d, mask_d, ident_d],
                reps=reps,
                stage=stage,
            )
        _split_waits(nc)
        _NC_CACHE[key] = nc
    return _NC_CACHE[key]


def host_prep(x, W):
    """Returns (wcr, xdg_all, xt_all, mask, ident); x-deriveds cover all B.
    Row order of the 128 K-rows is (i16, n): i = h*IP + i16."""
    bf = ml_dtypes.bfloat16
    nb = x.shape[0]
    # wcr[(i16*N + n), h, jm] = W[h*IP + i16, j, n, m]
    Wr = np.ascontiguousarray(W.transpose(0, 2, 1, 3)).reshape(I, N, JM)
    Wr = Wr.reshape(H, IP, N, JM)
    wcr = np.ascontiguousarray(Wr.transpose(1, 2, 0, 3)).reshape(128, H, JM)
    # x rows in the same (i16, n) order per h
    xr = x.reshape(nb, H, IP, N)
    xrows = np.ascontiguousarray(xr.transpose(2, 3, 1, 0)).reshape(128, H, nb)
    # zero-padded block-diagonal x operand rows
    rows = np.arange(128)
    i16_of_row = rows // N
    xblk = np.zeros((128, H, nb, IP), np.float32)
    for r in range(128):
        xblk[r, :, :, i16_of_row[r]] = xrows[r]
    mask = np.zeros((JM, J), np.float32)
    for j in range(J):
        mask[j * M : (j + 1) * M, j] = 1.0
    ident = np.eye(128, dtype=np.float32)
    return (
        wcr.astype(bf),
        xblk.astype(bf),
        xrows.astype(bf),
        mask.astype(bf),
        ident,
    )


def regroup(xblk_core):
    """xblk [128,H,nb,IP] -> [nb//8, T, 128, CH_T, 8, IP]."""
    nb = xblk_core.shape[2]
    xb = xblk_core.reshape(128, T, CH_T, nb, IP)
    xb = xb.transpose(3, 1, 0, 2, 4)  # [nb, T, 128, CH_T, IP]
    xb = xb.reshape(nb // 8, 8, T, 128, CH_T, IP).transpose(0, 2, 3, 4, 1, 5)
    return np.ascontiguousarray(xb)


def core_in_maps(x, W):
    """Per-core input dicts for run_bass_kernel_spmd."""
    wcr, xblk_all, xt_all, mask, ident = host_prep(x, W)
    in_maps = []
    for c in range(N_CORES):
        bs = slice(c * B_C, (c + 1) * B_C)
        in_maps.append(
            {
                "wcr": wcr,
                "xblk": regroup(xblk_all[:, :, bs, :]),
                "xt": np.ascontiguousarray(xt_all[:, :, bs]),
                "mask": mask,
                "ident": ident,
            }
        )
    return in_maps


def kernel(x, W):
    x = np.asarray(x, np.float32)
    W = np.asarray(W, np.float32)
    in_maps = core_in_maps(x, W)
    nc = _build_nc()
    res = run_bass_kernel_spmd(nc, in_maps, list(range(N_CORES)))
    out = np.concatenate([res.results[c]["v"] for c in range(N_CORES)], axis=0)
    return out.astype(np.float32)
